# revision 1
# baseline (speedup 1.0000x reference)
"""Trainium2 Bass kernel for a dense transformer block, distributed over 8
NeuronCores.

Sharding:
  phase 1 (attention): tensor-parallel over heads — each core computes 2 of
    the 16 heads end-to-end (QKV projections + causal softmax(QK^T)V), and
    returns the unnormalized per-head output O^T together with the softmax
    denominators (obtained via a ones-column appended to V).
  phase 2 (Wo + norms + FFN): data-parallel over tokens — each core handles
    512 of the 4096 token rows with replicated weights.

The host glues the phases: transposes x, normalizes/concats heads, and
re-shards tokens.  All matmuls run as float32r (full-rate fp32 PE mode).
"""

import math
from contextlib import ExitStack

import ml_dtypes
import numpy as np

BF_NP = ml_dtypes.bfloat16

import concourse.bass as bass
import concourse.mybir as mybir
import concourse.tile as tile
from concourse import bacc
from concourse.bass_utils import run_bass_kernel_spmd
from concourse.masks import make_identity, make_upper_triangular

FP = mybir.dt.float32
FPR = mybir.dt.float32r
BF = mybir.dt.bfloat16
AF = mybir.ActivationFunctionType

N_CORES = 8
P = 128
EPS = 1e-6

# exec times (ns) of the most recent kernel() call, one entry per phase, when
# tracing was enabled via BASS_TRACE=1; None entries otherwise.
LAST_EXEC_NS = []


def _fpr(ap):
    return ap.bitcast(FPR)


# --------------------------------------------------------------------------
# phase 1: per-core attention over a pair of heads
# --------------------------------------------------------------------------

def build_phase1(B, T, C, DH):
    HP = 2                      # heads per core
    DA = DH + 1                 # head dim + ones row (softmax denominator)
    NCC = C // P                # contraction chunks
    NT = T // P                 # key/value blocks of 128
    NQ = T // 512               # query chunks of 512
    NK = T // 1024              # query tiles of 1024
    scale = float(C) ** -0.5    # NOTE: reference scales by C**-0.5, not DH

    nc = bacc.Bacc("TRN2", debug=False)
    xT_d = nc.dram_tensor("xT", [B, C, T], BF, kind="ExternalInput").ap()
    wq_d = nc.dram_tensor("wq", [C, HP * DH], BF, kind="ExternalInput").ap()
    wk_d = nc.dram_tensor("wk", [C, HP * DH], BF, kind="ExternalInput").ap()
    wv_d = nc.dram_tensor("wv", [C, HP * DH], BF, kind="ExternalInput").ap()
    ot_d = nc.dram_tensor("ot", [B, HP, DA, T], FP, kind="ExternalOutput").ap()

    with tile.TileContext(nc) as tc, ExitStack() as ctx:
        const = ctx.enter_context(tc.tile_pool(name="const", bufs=1))
        xpool = ctx.enter_context(tc.tile_pool(name="xp", bufs=1))
        wpool = ctx.enter_context(tc.tile_pool(name="wp", bufs=1))
        qk_pool = ctx.enter_context(tc.tile_pool(name="qk", bufs=2))
        vt_pool = ctx.enter_context(tc.tile_pool(name="vtp", bufs=2))
        vaug_pool = ctx.enter_context(tc.tile_pool(name="vaug", bufs=2))
        pt_pool = ctx.enter_context(tc.tile_pool(name="pt", bufs=4))
        ot_pool = ctx.enter_context(tc.tile_pool(name="otp", bufs=2))

        # additive mask for the diagonal 128x128 block of S^T [s', q']:
        # 0 where q' >= s' (causal-valid), -1e30 where q' < s'
        negmask = const.tile([P, P], FP)
        nc.gpsimd.memset(negmask[:], 0.0)
        nc.gpsimd.affine_select(
            out=negmask[:], in_=negmask[:],
            compare_op=mybir.AluOpType.is_ge, fill=-1e30,
            base=0, pattern=[[1, P]], channel_multiplier=-1)
        ident = const.tile([P, P], BF)
        make_identity(nc, ident[:])
        ones_col = const.tile([P, NT * HP, 1], FP)
        nc.vector.memset(ones_col[:], 1.0)

        # weight chunks, loaded once
        wts = {}
        for name, src in (("q", wq_d), ("k", wk_d), ("v", wv_d)):
            wts[name] = []
            for c in range(NCC):
                t = wpool.tile([P, HP * DH], BF, tag=f"w{name}{c}")
                nc.sync.dma_start(out=t[:], in_=src[c * P:(c + 1) * P, :])
                wts[name].append(t)

        for b in range(B):
            xts = []
            for c in range(NCC):
                xt = xpool.tile([P, T], BF, tag=f"x{c}")
                nc.sync.dma_start(out=xt[:], in_=xT_d[b, c * P:(c + 1) * P, :])
                xts.append(xt)

            qt = qk_pool.tile([P, T], BF, tag="qt")
            kt = qk_pool.tile([P, T], BF, tag="kt")
            vaug = vaug_pool.tile([P, NT * HP, DA], BF, tag="vaug")
            # ones column per head-block (softmax denominator row of O^T)
            nc.vector.tensor_copy(vaug[:, :, DA - 1:DA], ones_col[:])

            with tc.tile_pool(name="proj_ps", bufs=3, space="PSUM") as proj_ps, \
                 tc.tile_pool(name="vt_ps", bufs=2, space="PSUM") as vt_ps:
                for wt, dst in ((wts["q"], qt), (wts["k"], kt)):
                    for n in range(NQ):
                        ps = proj_ps.tile([P, 512], FP, tag="proj")
                        for c in range(NCC):
                            nc.tensor.matmul(
                                ps[:], wt[c][:], xts[c][:, n * 512:(n + 1) * 512],
                                start=(c == 0), stop=(c == NCC - 1))
                        nc.vector.tensor_copy(dst[:, n * 512:(n + 1) * 512], ps[:])
                # V, then transpose into [s, d] layout with ones columns
                for n in range(NQ):
                    ps = proj_ps.tile([P, 512], FP, tag="proj")
                    for c in range(NCC):
                        nc.tensor.matmul(
                            ps[:], wts["v"][c][:], xts[c][:, n * 512:(n + 1) * 512],
                            start=(c == 0), stop=(c == NCC - 1))
                    vt = vt_pool.tile([P, 512], BF, tag="vt")
                    nc.vector.tensor_copy(vt[:], ps[:])
                    for u in range(4):
                        j = 4 * n + u
                        tp = vt_ps.tile([P, P], BF, tag="vtp")
                        nc.tensor.transpose(tp[:], vt[:, u * P:(u + 1) * P], ident[:])
                        nc.vector.tensor_copy(
                            vaug[:, j * HP, 0:DH], tp[:, 0:DH])
                        nc.vector.tensor_copy(
                            vaug[:, j * HP + 1, 0:DH], tp[:, DH:2 * DH])

            with tc.tile_pool(name="s_ps", bufs=2, space="PSUM") as s_ps, \
                 tc.tile_pool(name="o_ps", bufs=1, space="PSUM") as o_ps:
                ot_sbs = [ot_pool.tile([DA, T], FP, tag=f"ot{h}", name=f"ot{h}")
                          for h in range(HP)]
                for k in range(NK):
                    q_lo = 1024 * k
                    q_hi = 1024 * (k + 1)
                    o_tiles = [o_ps.tile([DA, 1024], FP, tag=f"o{h}", name=f"o{h}")
                               for h in range(HP)]
                    for j in range(8 * (k + 1)):
                        s0 = j * P
                        a0 = max(s0, q_lo)
                        # 512-grid chunks of the valid q range in this stripe
                        chunks = []
                        m0 = a0 // 512
                        for m in range(m0, q_hi // 512):
                            a = max(a0, m * 512)
                            e = (m + 1) * 512
                            chunks.append((a, e))
                        stl = [s_ps.tile([P, 1024], FP, tag="s", name="s")
                               for _ in range(HP)]
                        # emit head pairs adjacently: rows 0-63 (head A) and
                        # 64-127 (head B) run concurrently in the PE array
                        for (a, e) in chunks:
                            for h in range(HP):
                                hs = slice(h * DH, (h + 1) * DH)
                                nc.tensor.matmul(
                                    stl[h][:, a - q_lo:e - q_lo],
                                    kt[hs, s0:s0 + P], qt[hs, a:e],
                                    start=True, stop=True,
                                    tile_position=(h * DH, 0))
                        if q_lo <= s0:
                            for h in range(HP):
                                # diagonal block: additive causal mask
                                nc.vector.tensor_add(
                                    stl[h][:, s0 - q_lo:s0 - q_lo + P],
                                    stl[h][:, s0 - q_lo:s0 - q_lo + P],
                                    negmask[:])
                        for h in range(HP):
                            ptk = pt_pool.tile([P, 1024], BF, tag="pt")
                            nc.scalar.activation(
                                ptk[:, a0 - q_lo:1024], stl[h][:, a0 - q_lo:1024],
                                AF.Exp, scale=scale)
                            va = vaug[:, j * HP + h, :]
                            for (a, e) in chunks:
                                last_j = e // P - 1
                                nc.tensor.matmul(
                                    o_tiles[h][:, a - q_lo:e - q_lo],
                                    va, ptk[:, a - q_lo:e - q_lo],
                                    start=(j == 0), stop=(j == last_j))
                    for h in range(HP):
                        nc.vector.tensor_copy(
                            ot_sbs[h][:, q_lo:q_hi], o_tiles[h][:])
                for h in range(HP):
                    nc.sync.dma_start(out=ot_d[b, h], in_=ot_sbs[h][:])
    nc.compile()
    return nc


# --------------------------------------------------------------------------
# phase 2: per-core Wo projection + residual + rmsnorm + FFN + rmsnorm
# --------------------------------------------------------------------------

def build_phase2(NTOK, C, DFF):
    NTB = NTOK // P
    NCH = C // P
    NDF = DFF // P
    NG = DFF // 512
    halves = [(st, min(512, C - st)) for st in range(0, C, 512)]
    NH = len(halves)            # <=512-wide chunks of the channel dim

    nc = bacc.Bacc("TRN2", debug=False)
    xc_d = nc.dram_tensor("xc", [NTOK, C], FP, kind="ExternalInput").ap()
    at_d = nc.dram_tensor("attnT", [C, NTOK], BF, kind="ExternalInput").ap()
    wo_d = nc.dram_tensor("wo", [C, C], BF, kind="ExternalInput").ap()
    w1_d = nc.dram_tensor("w1", [C, DFF], BF, kind="ExternalInput").ap()
    w2_d = nc.dram_tensor("w2", [DFF, C], BF, kind="ExternalInput").ap()
    g1_d = nc.dram_tensor("g1", [C], FP, kind="ExternalInput").ap()
    g2_d = nc.dram_tensor("g2", [C], FP, kind="ExternalInput").ap()
    b1_d = nc.dram_tensor("b1", [DFF], FP, kind="ExternalInput").ap()
    b2_d = nc.dram_tensor("b2", [C], FP, kind="ExternalInput").ap()
    out_d = nc.dram_tensor("out", [NTOK, C], FP, kind="ExternalOutput").ap()

    def bcast_rows(src_ap, cols):
        # DRAM vector [cols] -> [P, cols] (same row in every partition)
        return bass.AP(tensor=src_ap.tensor, offset=src_ap.offset,
                       ap=[[0, P], [1, cols]])

    def col_ap(src_ap, start):
        # DRAM vector slice [start:start+P] -> [P, 1] (one value per partition)
        return bass.AP(tensor=src_ap.tensor, offset=src_ap.offset + start,
                       ap=[[1, P], [0, 1]])

    with tile.TileContext(nc) as tc, ExitStack() as ctx:
        const = ctx.enter_context(tc.tile_pool(name="const", bufs=1))
        work = ctx.enter_context(tc.tile_pool(name="work", bufs=2))
        stats = ctx.enter_context(tc.tile_pool(name="stats", bufs=4))
        h_pool = ctx.enter_context(tc.tile_pool(name="hp", bufs=1))
        ht_pool = ctx.enter_context(tc.tile_pool(name="htp", bufs=1))
        at_pool = ctx.enter_context(tc.tile_pool(name="atp", bufs=1))

        ident = const.tile([P, P], FP)
        make_identity(nc, ident[:])
        eps_t = const.tile([P, 1], FP)
        nc.vector.memset(eps_t[:], EPS)
        g1b = const.tile([P, C], FP)
        nc.sync.dma_start(out=g1b[:], in_=bcast_rows(g1_d, C))
        g2b = const.tile([P, C], FP)
        nc.sync.dma_start(out=g2b[:], in_=bcast_rows(g2_d, C))
        b2b = const.tile([P, C], FP)
        nc.sync.dma_start(out=b2b[:], in_=bcast_rows(b2_d, C))
        b1s = []
        for d in range(NDF):
            t = const.tile([P, 1], FP, tag=f"b1_{d}")
            nc.sync.dma_start(out=t[:], in_=col_ap(b1_d, d * P))
            b1s.append(t)

        def rmsnorm(src, gb, out_tag):
            sq = work.tile([P, C], FP, tag="sq")
            ssum = stats.tile([P, 1], FP, tag="ssum")
            nc.scalar.activation(sq[:], src[:], AF.Square, accum_out=ssum[:])
            rstd = stats.tile([P, 1], FP, tag="rstd")
            nc.scalar.activation(rstd[:], ssum[:], AF.Sqrt,
                                 scale=1.0 / C, bias=eps_t[:])
            rinv = stats.tile([P, 1], FP, tag="rinv")
            nc.vector.reciprocal(rinv[:], rstd[:])
            out = work.tile([P, C], FP, tag=out_tag)
            nc.vector.tensor_scalar_mul(out[:], src[:], rinv[:])
            nc.vector.tensor_mul(out[:], out[:], gb[:])
            return out

        # ---- stage 0: o = attnT^T @ Wo; r1 = x + o; h = rmsnorm(r1)*g1
        hs = []
        with tc.tile_pool(name="o_ps", bufs=1, space="PSUM") as o_ps, \
             tc.tile_pool(name="wop", bufs=NCH) as wop, \
             tc.tile_pool(name="atsp", bufs=NCH) as atsp, \
             tc.tile_pool(name="xcp", bufs=1) as xcp:
            atts, wots = [], []
            for c in range(NCH):
                att = atsp.tile([P, NTOK], BF, tag="at", name="at")
                nc.sync.dma_start(out=att[:], in_=at_d[c * P:(c + 1) * P, :])
                wot = wop.tile([P, C], BF, tag="wo", name="wo")
                nc.sync.dma_start(out=wot[:], in_=wo_d[c * P:(c + 1) * P, :])
                atts.append(att)
                wots.append(wot)
            xcs = []
            for tb in range(NTB):
                t = xcp.tile([P, C], FP, tag=f"xc{tb}")
                nc.sync.dma_start(out=t[:], in_=xc_d[tb * P:(tb + 1) * P, :])
                xcs.append(t)
            o_tiles = [o_ps.tile([P, 512], FP, tag=f"ops{i}", name=f"ops{i}")
                       for i in range(NTB * NH)]
            for c in range(NCH):
                att = atts[c]
                wot = wots[c]
                for tb in range(NTB):
                    for half, (hst, hw) in enumerate(halves):
                        nc.tensor.matmul(
                            o_tiles[tb * NH + half][:, :hw],
                            att[:, tb * P:(tb + 1) * P],
                            wot[:, hst:hst + hw],
                            start=(c == 0), stop=(c == NCH - 1))
            for tb in range(NTB):
                r1 = work.tile([P, C], FP, tag="r1")
                for half, (hst, hw) in enumerate(halves):
                    nc.vector.tensor_add(
                        r1[:, hst:hst + hw],
                        o_tiles[tb * NH + half][:, :hw],
                        xcs[tb][:, hst:hst + hw])
                hn = rmsnorm(r1, g1b, "hn")
                h = h_pool.tile([P, C], FP, tag=f"h{tb}")
                nc.vector.tensor_copy(h[:], hn[:])
                hs.append(h)

        # ---- stage 1: hT
        hts = [ht_pool.tile([P, NTOK], BF, tag=f"ht{c}", name=f"ht{c}")
               for c in range(NCH)]
        with tc.tile_pool(name="t_ps", bufs=4, space="PSUM") as t_ps:
            for tb in range(NTB):
                for c in range(NCH):
                    tp = t_ps.tile([P, P], FP, tag="tp")
                    nc.tensor.transpose(
                        tp[:], hs[tb][:, c * P:(c + 1) * P], ident[:])
                    nc.vector.tensor_copy(hts[c][:, tb * P:(tb + 1) * P], tp[:])

        # ---- stage 2: aT = silu(W1^T @ h^T + b1)
        ats = []
        w2p = ctx.enter_context(tc.tile_pool(name="w2p", bufs=5))
        with tc.tile_pool(name="a_ps", bufs=8, space="PSUM") as a_ps, \
             tc.tile_pool(name="w1p", bufs=5) as w1p, \
             tc.tile_pool(name="sgp", bufs=3) as sgp:
            for g in range(NG):
                aps = [a_ps.tile([P, NTOK], FP, tag="a", name="a") for _ in range(4)]
                for c in range(NCH):
                    w1t = w1p.tile([P, 512], BF, tag="w1")
                    nc.sync.dma_start(
                        out=w1t[:],
                        in_=w1_d[c * P:(c + 1) * P, g * 512:(g + 1) * 512])
                    for u in range(4):
                        nc.tensor.matmul(
                            aps[u][:], w1t[:, u * P:(u + 1) * P],
                            hts[c][:],
                            start=(c == 0), stop=(c == NCH - 1))
                for u in range(4):
                    d = 4 * g + u
                    sg = sgp.tile([P, NTOK], FP, tag="sg")
                    nc.scalar.activation(sg[:], aps[u][:], AF.Sigmoid,
                                         bias=b1s[d][:], scale=1.0)
                    at_t = at_pool.tile([P, NTOK], BF, tag=f"at{d}")
                    # silu(z) for z = a + b1: (a + b1) * sigmoid(a + b1)
                    nc.vector.scalar_tensor_tensor(
                        at_t[:], aps[u][:], b1s[d][:], sg[:],
                        op0=mybir.AluOpType.add, op1=mybir.AluOpType.mult)
                    ats.append(at_t)

        # ---- stage 3: f = aT^T @ W2; r2 = h + b2 + f; out = rmsnorm(r2)*g2
        with tc.tile_pool(name="f_ps", bufs=1, space="PSUM") as f_ps:
            fts = [f_ps.tile([P, 512], FP, tag=f"f{i}", name=f"f{i}")
                   for i in range(NTB * NH)]
            for d in range(NDF):
                w2t = w2p.tile([P, C], BF, tag="w2")
                nc.sync.dma_start(out=w2t[:], in_=w2_d[d * P:(d + 1) * P, :])
                for tb in range(NTB):
                    for half, (hst, hw) in enumerate(halves):
                        nc.tensor.matmul(
                            fts[tb * NH + half][:, :hw],
                            ats[d][:, tb * P:(tb + 1) * P],
                            w2t[:, hst:hst + hw],
                            start=(d == 0), stop=(d == NDF - 1))
            for tb in range(NTB):
                hb = work.tile([P, C], FP, tag="hb")
                nc.vector.tensor_add(hb[:], hs[tb][:], b2b[:])
                r2 = work.tile([P, C], FP, tag="r2")
                for half, (hst, hw) in enumerate(halves):
                    nc.vector.tensor_add(
                        r2[:, hst:hst + hw],
                        fts[tb * NH + half][:, :hw],
                        hb[:, hst:hst + hw])
                o = rmsnorm(r2, g2b, "outt")
                nc.sync.dma_start(out=out_d[tb * P:(tb + 1) * P, :], in_=o[:])
    nc.compile()
    return nc


# --------------------------------------------------------------------------
# host orchestration
# --------------------------------------------------------------------------

_CACHE = {}


def _phase1(B, T, C, DH):
    key = ("p1", B, T, C, DH)
    if key not in _CACHE:
        _CACHE[key] = build_phase1(B, T, C, DH)
    return _CACHE[key]


def _phase2(NTOK, C, DFF):
    key = ("p2", NTOK, C, DFF)
    if key not in _CACHE:
        _CACHE[key] = build_phase2(NTOK, C, DFF)
    return _CACHE[key]


def _run(nc, in_maps):
    import os
    trace = bool(os.environ.get("KERNEL_TRACE"))
    res = run_bass_kernel_spmd(nc, in_maps, core_ids=list(range(N_CORES)),
                               trace=trace)
    LAST_EXEC_NS.append(res.exec_time_ns)
    return res.results


def kernel(x, Wq, Wk, Wv, Wo, bo, W1, b1, W2, b2, g1, g2):
    f32 = lambda a: np.ascontiguousarray(np.asarray(a), dtype=np.float32)
    x = f32(x)
    Wq, Wk, Wv, Wo, bo = f32(Wq), f32(Wk), f32(Wv), f32(Wo), f32(bo)
    W1, b1, W2, b2, g1, g2 = f32(W1), f32(b1), f32(W2), f32(b2), f32(g1), f32(g2)

    B, T, C = x.shape
    H, _, DH = Wq.shape
    HP = H // N_CORES           # heads per core (2)
    DA = DH + 1
    LAST_EXEC_NS.clear()

    # ---- phase 1
    nc1 = _phase1(B, T, C, DH)
    xT = np.ascontiguousarray(x.transpose(0, 2, 1)).astype(BF_NP)
    in1 = []
    for i in range(N_CORES):
        pq = Wq[HP * i:HP * (i + 1)].transpose(1, 0, 2).reshape(C, HP * DH)
        pk = Wk[HP * i:HP * (i + 1)].transpose(1, 0, 2).reshape(C, HP * DH)
        pv = Wv[HP * i:HP * (i + 1)].transpose(1, 0, 2).reshape(C, HP * DH)
        in1.append({"xT": xT,
                    "wq": np.ascontiguousarray(pq).astype(BF_NP),
                    "wk": np.ascontiguousarray(pk).astype(BF_NP),
                    "wv": np.ascontiguousarray(pv).astype(BF_NP)})
    res1 = _run(nc1, in1)

    attn = np.empty((B, T, C), np.float32)
    for i in range(N_CORES):
        ot = res1[i]["ot"]                    # [B, HP, DA, T]
        o = ot[:, :, :DH, :]
        den = ot[:, :, DH, :]
        on = o / den[:, :, None, :]
        for hh in range(HP):
            hcol = (HP * i + hh) * DH
            attn[:, :, hcol:hcol + DH] = on[:, hh].transpose(0, 2, 1)

    # ---- phase 2
    NTOK = B * T // N_CORES
    nc2 = _phase2(NTOK, C, W1.shape[1])
    xf = x.reshape(B * T, C) + bo             # fold bo into the residual
    af = attn.reshape(B * T, C)
    wo_bf = Wo.astype(BF_NP)
    w1_bf = W1.astype(BF_NP)
    w2_bf = W2.astype(BF_NP)
    in2 = []
    for k in range(N_CORES):
        sl = slice(k * NTOK, (k + 1) * NTOK)
        in2.append({
            "xc": np.ascontiguousarray(xf[sl]),
            "attnT": np.ascontiguousarray(af[sl].T).astype(BF_NP),
            "wo": wo_bf, "w1": w1_bf, "w2": w2_bf,
            "g1": g1, "g2": g2, "b1": b1, "b2": b2,
        })
    res2 = _run(nc2, in2)
    out = np.concatenate([res2[k]["out"] for k in range(N_CORES)], axis=0)
    return out.reshape(B, T, C)



# revision 3
# speedup vs baseline: 1.0212x; 1.0212x over previous
"""Trainium2 Bass kernel for a dense transformer block, distributed over 8
NeuronCores.

Sharding:
  phase 1 (attention): tensor-parallel over heads — each core computes 2 of
    the 16 heads end-to-end (QKV projections + causal softmax(QK^T)V), and
    returns the unnormalized per-head output O^T together with the softmax
    denominators (obtained via a ones-column appended to V).
  phase 2 (Wo + norms + FFN): data-parallel over tokens — each core handles
    512 of the 4096 token rows with replicated weights.

The host glues the phases: transposes x, normalizes/concats heads, and
re-shards tokens.  All matmuls run as float32r (full-rate fp32 PE mode).
"""

import math
from contextlib import ExitStack

import ml_dtypes
import numpy as np

BF_NP = ml_dtypes.bfloat16

import concourse.bass as bass
import concourse.mybir as mybir
import concourse.tile as tile
from concourse import bacc
from concourse.bass_utils import run_bass_kernel_spmd
from concourse.masks import make_identity, make_upper_triangular

FP = mybir.dt.float32
FPR = mybir.dt.float32r
BF = mybir.dt.bfloat16
AF = mybir.ActivationFunctionType

N_CORES = 8
P = 128
EPS = 1e-6

# exec times (ns) of the most recent kernel() call, one entry per phase, when
# tracing was enabled via BASS_TRACE=1; None entries otherwise.
LAST_EXEC_NS = []
LAST_TRACES = []


def _fpr(ap):
    return ap.bitcast(FPR)


# --------------------------------------------------------------------------
# phase 1: per-core attention over a pair of heads
# --------------------------------------------------------------------------

def build_phase1(B, T, C, DH):
    HP = 2                      # heads per core
    DA = DH + 1                 # head dim + ones row (softmax denominator)
    NCC = C // P                # contraction chunks
    NT = T // P                 # key/value blocks of 128
    NQ = T // 512               # query chunks of 512
    NK = T // 1024              # query tiles of 1024
    scale = float(C) ** -0.5    # NOTE: reference scales by C**-0.5, not DH

    nc = bacc.Bacc("TRN2", debug=False)
    xT_d = nc.dram_tensor("xT", [B, C, T], BF, kind="ExternalInput").ap()
    wq_d = nc.dram_tensor("wq", [C, HP * DH], BF, kind="ExternalInput").ap()
    wk_d = nc.dram_tensor("wk", [C, HP * DH], BF, kind="ExternalInput").ap()
    wv_d = nc.dram_tensor("wv", [C, HP * DH], BF, kind="ExternalInput").ap()
    ot_d = nc.dram_tensor("ot", [B, HP, DA, T], FP, kind="ExternalOutput").ap()

    with tile.TileContext(nc) as tc, ExitStack() as ctx:
        const = ctx.enter_context(tc.tile_pool(name="const", bufs=1))
        xpool = ctx.enter_context(tc.tile_pool(name="xp", bufs=1))
        wpool = ctx.enter_context(tc.tile_pool(name="wp", bufs=1))
        qk_pool = ctx.enter_context(tc.tile_pool(name="qk", bufs=2))
        vt_pool = ctx.enter_context(tc.tile_pool(name="vtp", bufs=2))
        vaug_pool = ctx.enter_context(tc.tile_pool(name="vaug", bufs=2))
        pt_pool = ctx.enter_context(tc.tile_pool(name="pt", bufs=4))
        ot_pool = ctx.enter_context(tc.tile_pool(name="otp", bufs=2))

        # additive mask for the diagonal 128x128 block of S^T [s', q']:
        # 0 where q' >= s' (causal-valid), -1e30 where q' < s'
        negmask = const.tile([P, P], FP)
        nc.gpsimd.memset(negmask[:], 0.0)
        nc.gpsimd.affine_select(
            out=negmask[:], in_=negmask[:],
            compare_op=mybir.AluOpType.is_ge, fill=-1e30,
            base=0, pattern=[[1, P]], channel_multiplier=-1)
        ident = const.tile([P, P], BF)
        make_identity(nc, ident[:])
        ones_col = const.tile([P, NT * HP, 1], FP)
        nc.vector.memset(ones_col[:], 1.0)

        # weight chunks, loaded once
        wts = {}
        for name, src in (("q", wq_d), ("k", wk_d), ("v", wv_d)):
            wts[name] = []
            for c in range(NCC):
                t = wpool.tile([P, HP * DH], BF, tag=f"w{name}{c}")
                nc.sync.dma_start(out=t[:], in_=src[c * P:(c + 1) * P, :])
                wts[name].append(t)

        for b in range(B):
            xts = []
            for c in range(NCC):
                xt = xpool.tile([P, T], BF, tag=f"x{c}")
                nc.sync.dma_start(out=xt[:], in_=xT_d[b, c * P:(c + 1) * P, :])
                xts.append(xt)

            qt = qk_pool.tile([P, T], BF, tag="qt")
            kt = qk_pool.tile([P, T], BF, tag="kt")
            vaug = vaug_pool.tile([P, NT * HP, DA], BF, tag="vaug")
            # ones column per head-block (softmax denominator row of O^T)
            nc.vector.tensor_copy(vaug[:, :, DA - 1:DA], ones_col[:])

            with tc.tile_pool(name="proj_ps", bufs=3, space="PSUM") as proj_ps, \
                 tc.tile_pool(name="vt_ps", bufs=2, space="PSUM") as vt_ps:
                for wt, dst in ((wts["q"], qt), (wts["k"], kt)):
                    for n in range(NQ):
                        ps = proj_ps.tile([P, 512], FP, tag="proj")
                        for c in range(NCC):
                            nc.tensor.matmul(
                                ps[:], wt[c][:], xts[c][:, n * 512:(n + 1) * 512],
                                start=(c == 0), stop=(c == NCC - 1))
                        nc.vector.tensor_copy(dst[:, n * 512:(n + 1) * 512], ps[:])
                # V, then transpose into [s, d] layout with ones columns
                for n in range(NQ):
                    ps = proj_ps.tile([P, 512], FP, tag="proj")
                    for c in range(NCC):
                        nc.tensor.matmul(
                            ps[:], wts["v"][c][:], xts[c][:, n * 512:(n + 1) * 512],
                            start=(c == 0), stop=(c == NCC - 1))
                    vt = vt_pool.tile([P, 512], BF, tag="vt")
                    nc.vector.tensor_copy(vt[:], ps[:])
                    for u in range(4):
                        j = 4 * n + u
                        tp = vt_ps.tile([P, P], BF, tag="vtp")
                        nc.tensor.transpose(tp[:], vt[:, u * P:(u + 1) * P], ident[:])
                        nc.vector.tensor_copy(
                            vaug[:, j * HP, 0:DH], tp[:, 0:DH])
                        nc.vector.tensor_copy(
                            vaug[:, j * HP + 1, 0:DH], tp[:, DH:2 * DH])

            with tc.tile_pool(name="s_ps", bufs=2, space="PSUM") as s_ps, \
                 tc.tile_pool(name="o_ps", bufs=1, space="PSUM") as o_ps:
                ot_sbs = [ot_pool.tile([DA, T], FP, tag=f"ot{h}", name=f"ot{h}")
                          for h in range(HP)]
                for k in range(NK):
                    q_lo = 1024 * k
                    q_hi = 1024 * (k + 1)
                    o_tiles = [o_ps.tile([DA, 1024], FP, tag=f"o{h}", name=f"o{h}")
                               for h in range(HP)]
                    for j in range(8 * (k + 1)):
                        s0 = j * P
                        a0 = max(s0, q_lo)
                        # 512-grid chunks of the valid q range in this stripe
                        chunks = []
                        m0 = a0 // 512
                        for m in range(m0, q_hi // 512):
                            a = max(a0, m * 512)
                            e = (m + 1) * 512
                            chunks.append((a, e))
                        stl = [s_ps.tile([P, 1024], FP, tag="s", name="s")
                               for _ in range(HP)]
                        # emit head pairs adjacently: rows 0-63 (head A) and
                        # 64-127 (head B) run concurrently in the PE array
                        for (a, e) in chunks:
                            for h in range(HP):
                                hs = slice(h * DH, (h + 1) * DH)
                                nc.tensor.matmul(
                                    stl[h][:, a - q_lo:e - q_lo],
                                    kt[hs, s0:s0 + P], qt[hs, a:e],
                                    start=True, stop=True,
                                    tile_position=(h * DH, 0))
                        if q_lo <= s0:
                            for h in range(HP):
                                # diagonal block: additive causal mask
                                nc.vector.tensor_add(
                                    stl[h][:, s0 - q_lo:s0 - q_lo + P],
                                    stl[h][:, s0 - q_lo:s0 - q_lo + P],
                                    negmask[:])
                        for h in range(HP):
                            ptk = pt_pool.tile([P, 1024], BF, tag="pt")
                            nc.scalar.activation(
                                ptk[:, a0 - q_lo:1024], stl[h][:, a0 - q_lo:1024],
                                AF.Exp, scale=scale)
                            va = vaug[:, j * HP + h, :]
                            for (a, e) in chunks:
                                last_j = e // P - 1
                                nc.tensor.matmul(
                                    o_tiles[h][:, a - q_lo:e - q_lo],
                                    va, ptk[:, a - q_lo:e - q_lo],
                                    start=(j == 0), stop=(j == last_j))
                    for h in range(HP):
                        nc.vector.tensor_copy(
                            ot_sbs[h][:, q_lo:q_hi], o_tiles[h][:])
                for h in range(HP):
                    nc.sync.dma_start(out=ot_d[b, h], in_=ot_sbs[h][:])
    nc.compile()
    return nc


# --------------------------------------------------------------------------
# phase 2: per-core Wo projection + residual + rmsnorm + FFN + rmsnorm
# --------------------------------------------------------------------------

def build_phase2(NTOK, C, DFF):
    NTB = NTOK // P
    NCH = C // P
    NDF = DFF // P
    NG = DFF // 512
    halves = [(st, min(512, C - st)) for st in range(0, C, 512)]
    NH = len(halves)            # <=512-wide chunks of the channel dim

    nc = bacc.Bacc("TRN2", debug=False)
    xc_d = nc.dram_tensor("xc", [NTOK, C], FP, kind="ExternalInput").ap()
    at_d = nc.dram_tensor("attnT", [C, NTOK], BF, kind="ExternalInput").ap()
    wo_d = nc.dram_tensor("wo", [C, C], BF, kind="ExternalInput").ap()
    w1_d = nc.dram_tensor("w1", [C, DFF], BF, kind="ExternalInput").ap()
    w2_d = nc.dram_tensor("w2", [DFF, C], BF, kind="ExternalInput").ap()
    g1_d = nc.dram_tensor("g1", [C], FP, kind="ExternalInput").ap()
    g2_d = nc.dram_tensor("g2", [C], FP, kind="ExternalInput").ap()
    b1_d = nc.dram_tensor("b1", [DFF], FP, kind="ExternalInput").ap()
    b2_d = nc.dram_tensor("b2", [C], FP, kind="ExternalInput").ap()
    out_d = nc.dram_tensor("out", [NTOK, C], FP, kind="ExternalOutput").ap()

    def bcast_rows(src_ap, cols):
        # DRAM vector [cols] -> [P, cols] (same row in every partition)
        return bass.AP(tensor=src_ap.tensor, offset=src_ap.offset,
                       ap=[[0, P], [1, cols]])

    def col_ap(src_ap, start):
        # DRAM vector slice [start:start+P] -> [P, 1] (one value per partition)
        return bass.AP(tensor=src_ap.tensor, offset=src_ap.offset + start,
                       ap=[[1, P], [0, 1]])

    with tile.TileContext(nc) as tc, ExitStack() as ctx:
        const = ctx.enter_context(tc.tile_pool(name="const", bufs=1))
        work = ctx.enter_context(tc.tile_pool(name="work", bufs=2))
        stats = ctx.enter_context(tc.tile_pool(name="stats", bufs=4))
        h_pool = ctx.enter_context(tc.tile_pool(name="hp", bufs=1))
        ht_pool = ctx.enter_context(tc.tile_pool(name="htp", bufs=1))
        at_pool = ctx.enter_context(tc.tile_pool(name="atp", bufs=1))

        ident = const.tile([P, P], FP)
        make_identity(nc, ident[:])
        eps_t = const.tile([P, 1], FP)
        nc.vector.memset(eps_t[:], EPS)
        g1b = const.tile([P, C], FP)
        nc.sync.dma_start(out=g1b[:], in_=bcast_rows(g1_d, C))
        g2b = const.tile([P, C], FP)
        nc.sync.dma_start(out=g2b[:], in_=bcast_rows(g2_d, C))
        b2b = const.tile([P, C], FP)
        nc.sync.dma_start(out=b2b[:], in_=bcast_rows(b2_d, C))
        b1s = []
        for d in range(NDF):
            t = const.tile([P, 1], FP, tag=f"b1_{d}")
            nc.sync.dma_start(out=t[:], in_=col_ap(b1_d, d * P))
            b1s.append(t)

        def rmsnorm(src, gb, out_tag):
            sq = work.tile([P, C], FP, tag="sq")
            ssum = stats.tile([P, 1], FP, tag="ssum")
            nc.scalar.activation(sq[:], src[:], AF.Square, accum_out=ssum[:])
            rstd = stats.tile([P, 1], FP, tag="rstd")
            nc.scalar.activation(rstd[:], ssum[:], AF.Sqrt,
                                 scale=1.0 / C, bias=eps_t[:])
            rinv = stats.tile([P, 1], FP, tag="rinv")
            nc.vector.reciprocal(rinv[:], rstd[:])
            out = work.tile([P, C], FP, tag=out_tag)
            nc.vector.tensor_scalar_mul(out[:], src[:], rinv[:])
            nc.vector.tensor_mul(out[:], out[:], gb[:])
            return out

        # ---- stage 0: o = attnT^T @ Wo; r1 = x + o; h = rmsnorm(r1)*g1
        hs = []
        with tc.tile_pool(name="o_ps", bufs=1, space="PSUM") as o_ps, \
             tc.tile_pool(name="wop", bufs=NCH) as wop, \
             tc.tile_pool(name="atsp", bufs=NCH) as atsp, \
             tc.tile_pool(name="xcp", bufs=1) as xcp:
            atts, wots = [], []
            for c in range(NCH):
                att = atsp.tile([P, NTOK], BF, tag="at", name="at")
                nc.sync.dma_start(out=att[:], in_=at_d[c * P:(c + 1) * P, :])
                wot = wop.tile([P, C], BF, tag="wo", name="wo")
                nc.sync.dma_start(out=wot[:], in_=wo_d[c * P:(c + 1) * P, :])
                atts.append(att)
                wots.append(wot)
            xcs = []
            for tb in range(NTB):
                t = xcp.tile([P, C], FP, tag=f"xc{tb}")
                nc.sync.dma_start(out=t[:], in_=xc_d[tb * P:(tb + 1) * P, :])
                xcs.append(t)
            o_tiles = [o_ps.tile([P, 512], FP, tag=f"ops{i}", name=f"ops{i}")
                       for i in range(NTB * NH)]
            for c in range(NCH):
                att = atts[c]
                wot = wots[c]
                for tb in range(NTB):
                    for half, (hst, hw) in enumerate(halves):
                        nc.tensor.matmul(
                            o_tiles[tb * NH + half][:, :hw],
                            att[:, tb * P:(tb + 1) * P],
                            wot[:, hst:hst + hw],
                            start=(c == 0), stop=(c == NCH - 1))
            for tb in range(NTB):
                r1 = work.tile([P, C], FP, tag="r1")
                for half, (hst, hw) in enumerate(halves):
                    nc.vector.tensor_add(
                        r1[:, hst:hst + hw],
                        o_tiles[tb * NH + half][:, :hw],
                        xcs[tb][:, hst:hst + hw])
                hn = rmsnorm(r1, g1b, "hn")
                h = h_pool.tile([P, C], FP, tag=f"h{tb}")
                nc.vector.tensor_copy(h[:], hn[:])
                hs.append(h)

        # ---- stage 1: hT
        hts = [ht_pool.tile([P, NTOK], BF, tag=f"ht{c}", name=f"ht{c}")
               for c in range(NCH)]
        with tc.tile_pool(name="t_ps", bufs=4, space="PSUM") as t_ps:
            for tb in range(NTB):
                for c in range(NCH):
                    tp = t_ps.tile([P, P], FP, tag="tp")
                    nc.tensor.transpose(
                        tp[:], hs[tb][:, c * P:(c + 1) * P], ident[:])
                    nc.vector.tensor_copy(hts[c][:, tb * P:(tb + 1) * P], tp[:])

        # ---- stage 2: aT = silu(W1^T @ h^T + b1)
        ats = []
        w2p = ctx.enter_context(tc.tile_pool(name="w2p", bufs=5))
        with tc.tile_pool(name="a_ps", bufs=8, space="PSUM") as a_ps, \
             tc.tile_pool(name="w1p", bufs=5) as w1p, \
             tc.tile_pool(name="sgp", bufs=3) as sgp:
            for g in range(NG):
                aps = [a_ps.tile([P, NTOK], FP, tag="a", name="a") for _ in range(4)]
                for c in range(NCH):
                    w1t = w1p.tile([P, 512], BF, tag="w1")
                    nc.sync.dma_start(
                        out=w1t[:],
                        in_=w1_d[c * P:(c + 1) * P, g * 512:(g + 1) * 512])
                    for u in range(4):
                        nc.tensor.matmul(
                            aps[u][:], w1t[:, u * P:(u + 1) * P],
                            hts[c][:],
                            start=(c == 0), stop=(c == NCH - 1))
                for u in range(4):
                    d = 4 * g + u
                    sg = sgp.tile([P, NTOK], FP, tag="sg")
                    nc.scalar.activation(sg[:], aps[u][:], AF.Sigmoid,
                                         bias=b1s[d][:], scale=1.0)
                    at_t = at_pool.tile([P, NTOK], BF, tag=f"at{d}")
                    # silu(z) for z = a + b1: (a + b1) * sigmoid(a + b1)
                    nc.vector.scalar_tensor_tensor(
                        at_t[:], aps[u][:], b1s[d][:], sg[:],
                        op0=mybir.AluOpType.add, op1=mybir.AluOpType.mult)
                    ats.append(at_t)

        # ---- stage 3: f = aT^T @ W2; r2 = h + b2 + f; out = rmsnorm(r2)*g2
        with tc.tile_pool(name="f_ps", bufs=1, space="PSUM") as f_ps:
            fts = [f_ps.tile([P, 512], FP, tag=f"f{i}", name=f"f{i}")
                   for i in range(NTB * NH)]
            for d in range(NDF):
                w2t = w2p.tile([P, C], BF, tag="w2")
                nc.sync.dma_start(out=w2t[:], in_=w2_d[d * P:(d + 1) * P, :])
                for tb in range(NTB):
                    for half, (hst, hw) in enumerate(halves):
                        nc.tensor.matmul(
                            fts[tb * NH + half][:, :hw],
                            ats[d][:, tb * P:(tb + 1) * P],
                            w2t[:, hst:hst + hw],
                            start=(d == 0), stop=(d == NDF - 1))
            for tb in range(NTB):
                hb = work.tile([P, C], FP, tag="hb")
                nc.vector.tensor_add(hb[:], hs[tb][:], b2b[:])
                r2 = work.tile([P, C], FP, tag="r2")
                for half, (hst, hw) in enumerate(halves):
                    nc.vector.tensor_add(
                        r2[:, hst:hst + hw],
                        fts[tb * NH + half][:, :hw],
                        hb[:, hst:hst + hw])
                o = rmsnorm(r2, g2b, "outt")
                nc.sync.dma_start(out=out_d[tb * P:(tb + 1) * P, :], in_=o[:])
    nc.compile()
    return nc


# --------------------------------------------------------------------------
# host orchestration
# --------------------------------------------------------------------------

_CACHE = {}


def _phase1(B, T, C, DH):
    key = ("p1", B, T, C, DH)
    if key not in _CACHE:
        _CACHE[key] = build_phase1(B, T, C, DH)
    return _CACHE[key]


def _phase2(NTOK, C, DFF):
    key = ("p2", NTOK, C, DFF)
    if key not in _CACHE:
        _CACHE[key] = build_phase2(NTOK, C, DFF)
    return _CACHE[key]


def _run(nc, in_maps):
    import os
    trace = bool(os.environ.get("KERNEL_TRACE"))
    res = run_bass_kernel_spmd(nc, in_maps, core_ids=list(range(N_CORES)),
                               trace=trace)
    LAST_EXEC_NS.append(res.exec_time_ns)
    LAST_TRACES.append(res.instructions_and_trace)
    return res.results


def kernel(x, Wq, Wk, Wv, Wo, bo, W1, b1, W2, b2, g1, g2):
    f32 = lambda a: np.ascontiguousarray(np.asarray(a), dtype=np.float32)
    x = f32(x)
    Wq, Wk, Wv, Wo, bo = f32(Wq), f32(Wk), f32(Wv), f32(Wo), f32(bo)
    W1, b1, W2, b2, g1, g2 = f32(W1), f32(b1), f32(W2), f32(b2), f32(g1), f32(g2)

    B, T, C = x.shape
    H, _, DH = Wq.shape
    HP = H // N_CORES           # heads per core (2)
    DA = DH + 1
    LAST_EXEC_NS.clear()

    # ---- phase 1
    nc1 = _phase1(B, T, C, DH)
    xT = np.ascontiguousarray(x.transpose(0, 2, 1)).astype(BF_NP)
    in1 = []
    for i in range(N_CORES):
        pq = Wq[HP * i:HP * (i + 1)].transpose(1, 0, 2).reshape(C, HP * DH)
        pk = Wk[HP * i:HP * (i + 1)].transpose(1, 0, 2).reshape(C, HP * DH)
        pv = Wv[HP * i:HP * (i + 1)].transpose(1, 0, 2).reshape(C, HP * DH)
        in1.append({"xT": xT,
                    "wq": np.ascontiguousarray(pq).astype(BF_NP),
                    "wk": np.ascontiguousarray(pk).astype(BF_NP),
                    "wv": np.ascontiguousarray(pv).astype(BF_NP)})
    res1 = _run(nc1, in1)

    attn = np.empty((B, T, C), np.float32)
    for i in range(N_CORES):
        ot = res1[i]["ot"]                    # [B, HP, DA, T]
        o = ot[:, :, :DH, :]
        den = ot[:, :, DH, :]
        on = o / den[:, :, None, :]
        for hh in range(HP):
            hcol = (HP * i + hh) * DH
            attn[:, :, hcol:hcol + DH] = on[:, hh].transpose(0, 2, 1)

    # ---- phase 2
    NTOK = B * T // N_CORES
    nc2 = _phase2(NTOK, C, W1.shape[1])
    xf = x.reshape(B * T, C) + bo             # fold bo into the residual
    af = attn.reshape(B * T, C)
    wo_bf = Wo.astype(BF_NP)
    w1_bf = W1.astype(BF_NP)
    w2_bf = W2.astype(BF_NP)
    in2 = []
    for k in range(N_CORES):
        sl = slice(k * NTOK, (k + 1) * NTOK)
        in2.append({
            "xc": np.ascontiguousarray(xf[sl]),
            "attnT": np.ascontiguousarray(af[sl].T).astype(BF_NP),
            "wo": wo_bf, "w1": w1_bf, "w2": w2_bf,
            "g1": g1, "g2": g2, "b1": b1, "b2": b2,
        })
    res2 = _run(nc2, in2)
    out = np.concatenate([res2[k]["out"] for k in range(N_CORES)], axis=0)
    return out.reshape(B, T, C)



# revision 10
# speedup vs baseline: 1.0945x; 1.0719x over previous
"""Trainium2 Bass kernel for a dense transformer block, distributed over 8
NeuronCores.

Sharding:
  phase 1 (attention): tensor-parallel over heads — each core computes 2 of
    the 16 heads end-to-end (QKV projections + causal softmax(QK^T)V), and
    returns the unnormalized per-head output O^T together with the softmax
    denominators (obtained via a ones-column appended to V).
  phase 2 (Wo + norms + FFN): data-parallel over tokens — each core handles
    512 of the 4096 token rows with replicated weights.

v2 notes (vs the first working version):
  - QKV projections pipeline against the x DMA stream (c-chunk outer loop).
  - Stripe phase software-pipelines S -> exp -> PV with PV one stripe
    behind, so the PE never waits on the scalar engine's exp.
  - Phase 2 runs Wo token-pair-wise against the weight DMA stream, rmsnorm
    per token block, silu directly on the scalar engine, W2 d-outer with
    residuals injected into PSUM via identity matmuls, per-block output DMA.
  - All matmuls bf16 (inputs quantized host-side); activation tables are
    warmed during initial DMA stalls.
"""

import math
from contextlib import ExitStack

import ml_dtypes
import numpy as np

BF_NP = ml_dtypes.bfloat16

import concourse.bass as bass
import concourse.mybir as mybir
import concourse.tile as tile
from concourse import bacc
from concourse.bass_utils import run_bass_kernel_spmd
from concourse.masks import make_identity, make_upper_triangular

FP = mybir.dt.float32
FPR = mybir.dt.float32r
BF = mybir.dt.bfloat16
AF = mybir.ActivationFunctionType

N_CORES = 8
P = 128
EPS = 1e-6

# exec times (ns) of the most recent kernel() call, one entry per phase, when
# tracing was enabled via BASS_TRACE=1; None entries otherwise.
LAST_EXEC_NS = []
LAST_TRACES = []


# --------------------------------------------------------------------------
# phase 1: per-core attention over a pair of heads
# --------------------------------------------------------------------------

def build_phase1(B, T, C, DH):
    HP = 2                      # heads per core
    DA = DH + 1                 # head dim + ones row (softmax denominator)
    NCC = C // P                # contraction chunks (8)
    NT = T // P                 # 128-token blocks (16)
    NQT = T // 1024             # 1024-wide q tiles per batch (2)
    scale = float(C) ** -0.5    # NOTE: reference scales by C**-0.5, not DH

    nc = bacc.Bacc("TRN2", debug=False)
    xT_d = nc.dram_tensor("xT", [B, C, T], BF, kind="ExternalInput").ap()
    wq_d = nc.dram_tensor("wq", [C, HP * DH], BF, kind="ExternalInput").ap()
    wk_d = nc.dram_tensor("wk", [C, HP * DH], BF, kind="ExternalInput").ap()
    wv_d = nc.dram_tensor("wv", [C, HP * DH], BF, kind="ExternalInput").ap()
    ot_d = nc.dram_tensor("ot", [B, HP, DA, T], BF, kind="ExternalOutput").ap()

    with tile.TileContext(nc) as tc, ExitStack() as ctx:
        const = ctx.enter_context(tc.tile_pool(name="const", bufs=1))
        xpool = ctx.enter_context(tc.tile_pool(name="xp", bufs=1))
        wpool = ctx.enter_context(tc.tile_pool(name="wp", bufs=1))
        qk_pool = ctx.enter_context(tc.tile_pool(name="qk", bufs=1))
        vaug_pool = ctx.enter_context(tc.tile_pool(name="vaug", bufs=1))
        vt_pool = ctx.enter_context(tc.tile_pool(name="vtp", bufs=2))
        pt_pool = ctx.enter_context(tc.tile_pool(name="pt", bufs=3))
        ot_pool = ctx.enter_context(tc.tile_pool(name="otp", bufs=4))

        # weights first (small), so QKV can start on the first x chunk
        wts = {}
        for name, src in (("q", wq_d), ("k", wk_d), ("v", wv_d)):
            wts[name] = []
            for c in range(NCC):
                t = wpool.tile([P, HP * DH], BF, tag=f"w{name}{c}")
                nc.sync.dma_start(out=t[:], in_=src[c * P:(c + 1) * P, :])
                wts[name].append(t)

        # x for both batches, c-major, so compute can chase the DMA stream
        xts = [[None] * NCC for _ in range(B)]
        for b in range(B):
            for c in range(NCC):
                t = xpool.tile([P, T], BF, tag=f"x{b}_{c}")
                nc.sync.dma_start(out=t[:], in_=xT_d[b, c * P:(c + 1) * P, :])
                xts[b][c] = t

        # warm the exp activation table during the DMA stall
        junk = const.tile([P, 1], FP)
        nc.vector.memset(junk[:], 0.0)
        junk2 = const.tile([P, 1], FP)
        nc.scalar.activation(junk2[:], junk[:], AF.Exp)

        # additive mask for the diagonal 128x128 block of S^T [s', q']:
        # 0 where q' >= s' (causal-valid), -1e30 where q' < s'
        negmask = const.tile([P, P], FP)
        nc.gpsimd.memset(negmask[:], 0.0)
        nc.gpsimd.affine_select(
            out=negmask[:], in_=negmask[:],
            compare_op=mybir.AluOpType.is_ge, fill=-1e30,
            base=0, pattern=[[1, P]], channel_multiplier=-1)
        ident = const.tile([P, P], BF)
        make_identity(nc, ident[:])
        ones_col = const.tile([P, NT * HP, 1], FP)
        nc.vector.memset(ones_col[:], 1.0)

        for b in range(B):
            qt = qk_pool.tile([P, T], BF, tag=f"qt{b}")
            kt = qk_pool.tile([P, T], BF, tag=f"kt{b}")
            vaug = vaug_pool.tile([P, NT * HP, DA], BF, tag=f"vaug{b}")
            nc.vector.tensor_copy(vaug[:, :, DA - 1:DA], ones_col[:])

            # ---- stage A: QKV projections (q/k c-outer against DMA stream)
            with tc.tile_pool(name="qk_ps", bufs=1, space="PSUM") as qk_ps:
                q_ps = qk_ps.tile([P, T], FP, tag="qps", name="qps")
                k_ps = qk_ps.tile([P, T], FP, tag="kps", name="kps")
                for c in range(NCC):
                    for wt, ps in ((wts["q"], q_ps), (wts["k"], k_ps)):
                        for n in range(T // 512):
                            nc.tensor.matmul(
                                ps[:, n * 512:(n + 1) * 512],
                                wt[c][:], xts[b][c][:, n * 512:(n + 1) * 512],
                                start=(c == 0), stop=(c == NCC - 1))
                for half in range(T // 1024):
                    sl = slice(half * 1024, (half + 1) * 1024)
                    nc.vector.tensor_copy(qt[:, sl], q_ps[:, sl])
                    nc.vector.tensor_copy(kt[:, sl], k_ps[:, sl])

            # V projection + transpose into [s, d] layout with ones column
            with tc.tile_pool(name="v_ps", bufs=2, space="PSUM") as v_ps, \
                 tc.tile_pool(name="tp_ps", bufs=2, space="PSUM") as tp_ps:
                for n8 in range(T // 512):
                    vps = v_ps.tile([P, 512], FP, tag="v")
                    for c in range(NCC):
                        nc.tensor.matmul(
                            vps[:], wts["v"][c][:],
                            xts[b][c][:, n8 * 512:(n8 + 1) * 512],
                            start=(c == 0), stop=(c == NCC - 1))
                    vt = vt_pool.tile([P, 512], BF, tag="vt")
                    nc.vector.tensor_copy(vt[:], vps[:])
                    tp = tp_ps.tile([P, 4, P], BF, tag="tp")
                    for u in range(4):
                        nc.tensor.transpose(
                            tp[:, u, :], vt[:, u * P:(u + 1) * P], ident[:])
                    # [tok, (j,h)-dh] -> vaug slots 8*n8 .. 8*n8+8 (64 of 65)
                    nc.vector.tensor_copy(
                        vaug[:, n8 * 8:(n8 + 1) * 8, 0:DH], tp[:])

            # ---- stage B: causal stripes, PV pipelined one stripe behind
            with tc.tile_pool(name="s_ps", bufs=1, space="PSUM") as s_ps, \
                 tc.tile_pool(name="o_ps", bufs=1, space="PSUM") as o_ps:
                for kq in range(NQT):
                    q_lo = 1024 * kq
                    q_hi = 1024 * (kq + 1)
                    o_t = [o_ps.tile([DA, 1024], FP, tag=f"o{h}", name=f"o{h}")
                           for h in range(HP)]
                    pend = []

                    def emit_pv(ent):
                        j, a0, chunks, ptks = ent
                        for h in range(HP):
                            va = vaug[:, j * HP + h, :]
                            for (a, e) in chunks:
                                last_j = e // P - 1
                                nc.tensor.matmul(
                                    o_t[h][:, a - q_lo:e - q_lo],
                                    va, ptks[h][:, a - q_lo:e - q_lo],
                                    start=(j == 0), stop=(j == last_j))

                    for j in range(8 * (kq + 1)):
                        s0 = j * P
                        a0 = max(s0, q_lo)
                        chunks = []
                        for m in range(a0 // 512, q_hi // 512):
                            a = max(a0, m * 512)
                            chunks.append((a, (m + 1) * 512))
                        stl = [s_ps.tile([P, 1024], FP, tag=f"s{h}", name=f"s{h}")
                               for h in range(HP)]
                        for (a, e) in chunks:
                            for h in range(HP):
                                hs = slice(h * DH, (h + 1) * DH)
                                nc.tensor.matmul(
                                    stl[h][:, a - q_lo:e - q_lo],
                                    kt[hs, s0:s0 + P], qt[hs, a:e],
                                    start=True, stop=True,
                                    tile_position=(h * DH, 0))
                        if q_lo <= s0:
                            for h in range(HP):
                                nc.vector.tensor_add(
                                    stl[h][:, s0 - q_lo:s0 - q_lo + P],
                                    stl[h][:, s0 - q_lo:s0 - q_lo + P],
                                    negmask[:])
                        ptks = []
                        for h in range(HP):
                            ptk = pt_pool.tile([P, 1024], BF, tag=f"pt{h}")
                            nc.scalar.activation(
                                ptk[:, a0 - q_lo:1024], stl[h][:, a0 - q_lo:1024],
                                AF.Exp, scale=scale)
                            ptks.append(ptk)
                        pend.append((j, a0, chunks, ptks))
                        if len(pend) >= 2:
                            emit_pv(pend.pop(0))
                    emit_pv(pend.pop(0))
                    for h in range(HP):
                        osb = ot_pool.tile([DA, 1024], BF, tag=f"ot{h}")
                        nc.vector.tensor_copy(osb[:], o_t[h][:])
                        nc.gpsimd.dma_start(
                            out=ot_d[b, h, :, q_lo:q_hi], in_=osb[:])
    nc.compile()
    return nc


# --------------------------------------------------------------------------
# phase 2: per-core Wo projection + residual + rmsnorm + FFN + rmsnorm
# --------------------------------------------------------------------------

def build_phase2(NTOK, C, DFF):
    NTB = NTOK // P             # 4 token blocks
    NCH = C // P                # 8 channel chunks
    NDF = DFF // P              # 32 dff chunks
    NG = DFF // 512             # 8 w1 groups

    nc = bacc.Bacc("TRN2", debug=False)
    xc_d = nc.dram_tensor("xc", [NTOK, C], BF, kind="ExternalInput").ap()
    at_d = nc.dram_tensor("attnT", [C, NTOK], BF, kind="ExternalInput").ap()
    wo_d = nc.dram_tensor("wo", [C, C], BF, kind="ExternalInput").ap()
    w1_d = nc.dram_tensor("w1", [C, DFF], BF, kind="ExternalInput").ap()
    w2_d = nc.dram_tensor("w2", [DFF, C], BF, kind="ExternalInput").ap()
    g1_d = nc.dram_tensor("g1", [C], FP, kind="ExternalInput").ap()
    g2_d = nc.dram_tensor("g2", [C], FP, kind="ExternalInput").ap()
    b1_d = nc.dram_tensor("b1", [DFF], FP, kind="ExternalInput").ap()
    b2_d = nc.dram_tensor("b2", [C], BF, kind="ExternalInput").ap()
    out_d = nc.dram_tensor("out", [NTOK, C], FP, kind="ExternalOutput").ap()

    def bcast_rows(src_ap, cols):
        # DRAM vector [cols] -> [P, cols] (same row in every partition)
        return bass.AP(tensor=src_ap.tensor, offset=src_ap.offset,
                       ap=[[0, P], [1, cols]])

    halves = ((0, 512), (512, 512))

    with tile.TileContext(nc) as tc, ExitStack() as ctx:
        const = ctx.enter_context(tc.tile_pool(name="const", bufs=1))
        work = ctx.enter_context(tc.tile_pool(name="work", bufs=2))
        stats = ctx.enter_context(tc.tile_pool(name="stats", bufs=4))
        h_pool = ctx.enter_context(tc.tile_pool(name="hp", bufs=1))
        at_pool = ctx.enter_context(tc.tile_pool(name="atp", bufs=1))
        out_pool = ctx.enter_context(tc.tile_pool(name="outp", bufs=2))

        # --- input DMAs, in consumption order (sync queue; outputs go on
        # the gpsimd queue so they never sit behind these)
        wo_po = ctx.enter_context(tc.tile_pool(name="wop", bufs=1))
        atts, wots = [], []
        for c in range(NCH):
            att = wo_po.tile([P, NTOK], BF, tag=f"at{c}")
            nc.sync.dma_start(out=att[:], in_=at_d[c * P:(c + 1) * P, :])
            wot = wo_po.tile([P, C], BF, tag=f"wo{c}")
            nc.sync.dma_start(out=wot[:], in_=wo_d[c * P:(c + 1) * P, :])
            atts.append(att)
            wots.append(wot)
        xcs = []
        for tb in range(NTB):
            t = wo_po.tile([P, C], BF, tag=f"xc{tb}")
            nc.sync.dma_start(out=t[:], in_=xc_d[tb * P:(tb + 1) * P, :])
            xcs.append(t)
        eps_t = const.tile([P, 1], FP)
        nc.vector.memset(eps_t[:], EPS)
        g1b = const.tile([P, C], FP)
        nc.sync.dma_start(out=g1b[:], in_=bcast_rows(g1_d, C))
        g2b = const.tile([P, C], FP)
        nc.sync.dma_start(out=g2b[:], in_=bcast_rows(g2_d, C))
        b2row = const.tile([1, C], BF)
        nc.sync.dma_start(
            out=b2row[:],
            in_=bass.AP(tensor=b2_d.tensor, offset=b2_d.offset,
                        ap=[[0, 1], [1, C]]))
        ones1 = const.tile([1, P], BF)
        nc.vector.memset(ones1[:], 1.0)
        # b1 as [P, NDF]: element (p, d) = b1[d*P + p]
        b1s = const.tile([P, NDF], FP)
        nc.sync.dma_start(
            out=b1s[:],
            in_=bass.AP(tensor=b1_d.tensor, offset=b1_d.offset,
                        ap=[[1, P], [P, NDF]]))
        # w1 chunks, streamed through a rotating 3-group window
        w1_po = ctx.enter_context(tc.tile_pool(name="w1p", bufs=3))
        w1sb = [[None] * NCH for _ in range(NG)]
        for g in range(NG):
            for c in range(NCH):
                t = w1_po.tile([P, 512], BF, tag=f"w1_{c}", name=f"w1_{c}")
                nc.sync.dma_start(
                    out=t[:],
                    in_=w1_d[c * P:(c + 1) * P, g * 512:(g + 1) * 512])
                w1sb[g][c] = t
        # w2 chunks, streamed through a rotating 8-chunk window
        w2_po = ctx.enter_context(tc.tile_pool(name="w2p", bufs=8))
        w2sb = []
        for d in range(NDF):
            t = w2_po.tile([P, C], BF, tag="w2", name="w2")
            nc.sync.dma_start(out=t[:], in_=w2_d[d * P:(d + 1) * P, :])
            w2sb.append(t)

        # warm the sqrt table during the DMA stall
        junk = const.tile([P, 1], FP)
        nc.vector.memset(junk[:], 1.0)
        junk2 = const.tile([P, 1], FP)
        nc.scalar.activation(junk2[:], junk[:], AF.Sqrt)
        ident = const.tile([P, P], BF)
        make_identity(nc, ident[:])

        hT = h_pool.tile([P, NCH, NTOK], BF, tag="hT")      # h^T, c-major
        h_bfs = []

        # ---- stage 0: o = attn@Wo + x (residual via identity matmul into
        # PSUM); rmsnorm per token block; transpose into hT
        with tc.tile_pool(name="o_ps", bufs=1, space="PSUM") as o_ps, \
             tc.tile_pool(name="t_ps", bufs=2, space="PSUM") as t_ps:
            for pair_i in range(NTB // 2):
                pair = (2 * pair_i, 2 * pair_i + 1)
                o2 = [[o_ps.tile([P, 512], FP, tag=f"o2_{si}_{hi}",
                                 name=f"o2_{si}_{hi}")
                       for hi in range(2)] for si in range(2)]
                for c in range(NCH):
                    for si, tb in enumerate(pair):
                        for hi, (hst, hw) in enumerate(halves):
                            nc.tensor.matmul(
                                o2[si][hi][:],
                                atts[c][:, tb * P:(tb + 1) * P],
                                wots[c][:, hst:hst + hw],
                                start=(c == 0), stop=False)
                for si, tb in enumerate(pair):
                    for hi, (hst, hw) in enumerate(halves):
                        nc.tensor.matmul(
                            o2[si][hi][:], ident[:], xcs[tb][:, hst:hst + hw],
                            start=False, stop=True)
                for si, tb in enumerate(pair):
                    sq = work.tile([P, 512], FP, tag="sq")
                    ss = [stats.tile([P, 1], FP, tag=f"ss{hi}", name=f"ss{hi}")
                          for hi in range(2)]
                    for hi in range(2):
                        nc.scalar.activation(sq[:], o2[si][hi][:], AF.Square,
                                             accum_out=ss[hi][:])
                    nc.vector.tensor_add(ss[0][:], ss[0][:], ss[1][:])
                    rstd = stats.tile([P, 1], FP, tag="rstd")
                    nc.scalar.activation(rstd[:], ss[0][:], AF.Sqrt,
                                         scale=1.0 / C, bias=eps_t[:])
                    rinv = stats.tile([P, 1], FP, tag="rinv")
                    nc.vector.reciprocal(rinv[:], rstd[:])
                    h_bf = h_pool.tile([P, C], BF, tag=f"h{tb}")
                    for hi, (hst, hw) in enumerate(halves):
                        nc.vector.scalar_tensor_tensor(
                            h_bf[:, hst:hst + hw], o2[si][hi][:], rinv[:],
                            g1b[:, hst:hst + hw],
                            op0=mybir.AluOpType.mult, op1=mybir.AluOpType.mult)
                    h_bfs.append(h_bf)
                    for g4 in range(2):
                        tp = t_ps.tile([P, 4, P], BF, tag="tp")
                        for u in range(4):
                            nc.tensor.transpose(
                                tp[:, u, :],
                                h_bf[:, (g4 * 4 + u) * P:(g4 * 4 + u + 1) * P],
                                ident[:])
                        nc.vector.tensor_copy(
                            hT[:, g4 * 4:(g4 + 1) * 4, tb * P:(tb + 1) * P],
                            tp[:])

        # ---- stage 1: aT = silu(W1^T @ h^T + b1), directly via scalar silu
        ats = []
        with tc.tile_pool(name="a_ps", bufs=2, space="PSUM") as a_ps:
            for g in range(NG):
                aps = [a_ps.tile([P, NTOK], FP, tag=f"a{u}", name=f"a{u}")
                       for u in range(4)]
                for c in range(NCH):
                    for u in range(4):
                        nc.tensor.matmul(
                            aps[u][:], w1sb[g][c][:, u * P:(u + 1) * P],
                            hT[:, c, :],
                            start=(c == 0), stop=(c == NCH - 1))
                for u in range(4):
                    d = 4 * g + u
                    at_t = at_pool.tile([P, NTOK], BF, tag=f"at{d}")
                    nc.scalar.activation(at_t[:], aps[u][:], AF.Silu,
                                         bias=b1s[:, d:d + 1], scale=1.0)
                    ats.append(at_t)

        # ---- stage 2: f = aT^T @ W2 + h + b2 (residuals via identity/ones
        # matmuls); d-outer so w2 streams; rmsnorm + store per token block
        with tc.tile_pool(name="f_ps", bufs=1, space="PSUM") as f_ps:
            f2 = [f_ps.tile([P, C], FP, tag=f"f{tb}", name=f"f{tb}")
                  for tb in range(NTB)]
            for tb in range(NTB):
                for (hst, hw) in halves:
                    nc.tensor.matmul(
                        f2[tb][:, hst:hst + hw], ident[:],
                        h_bfs[tb][:, hst:hst + hw],
                        start=True, stop=False)
                    nc.tensor.matmul(
                        f2[tb][:, hst:hst + hw], ones1[:],
                        b2row[:, hst:hst + hw],
                        start=False, stop=False)
            for d in range(NDF):
                for tb in range(NTB):
                    for (hst, hw) in halves:
                        nc.tensor.matmul(
                            f2[tb][:, hst:hst + hw],
                            ats[d][:, tb * P:(tb + 1) * P],
                            w2sb[d][:, hst:hst + hw],
                            start=False, stop=(d == NDF - 1))
            for tb in range(NTB):
                sq = work.tile([P, C], FP, tag="sq2")
                ssum = stats.tile([P, 1], FP, tag="ssum2")
                nc.scalar.activation(sq[:], f2[tb][:], AF.Square,
                                     accum_out=ssum[:])
                rstd = stats.tile([P, 1], FP, tag="rstd2")
                nc.scalar.activation(rstd[:], ssum[:], AF.Sqrt,
                                     scale=1.0 / C, bias=eps_t[:])
                rinv = stats.tile([P, 1], FP, tag="rinv2")
                nc.vector.reciprocal(rinv[:], rstd[:])
                o = out_pool.tile([P, C], FP, tag="outt")
                nc.vector.scalar_tensor_tensor(
                    o[:], f2[tb][:], rinv[:], g2b[:],
                    op0=mybir.AluOpType.mult, op1=mybir.AluOpType.mult)
                nc.gpsimd.dma_start(
                    out=out_d[tb * P:(tb + 1) * P, :], in_=o[:])
    nc.compile()
    return nc


# --------------------------------------------------------------------------
# host orchestration
# --------------------------------------------------------------------------

_CACHE = {}


def _phase1(B, T, C, DH):
    key = ("p1", B, T, C, DH)
    if key not in _CACHE:
        _CACHE[key] = build_phase1(B, T, C, DH)
    return _CACHE[key]


def _phase2(NTOK, C, DFF):
    key = ("p2", NTOK, C, DFF)
    if key not in _CACHE:
        _CACHE[key] = build_phase2(NTOK, C, DFF)
    return _CACHE[key]


def _run(nc, in_maps):
    import os
    trace = bool(os.environ.get("KERNEL_TRACE"))
    res = run_bass_kernel_spmd(nc, in_maps, core_ids=list(range(N_CORES)),
                               trace=trace)
    LAST_EXEC_NS.append(res.exec_time_ns)
    LAST_TRACES.append(res.instructions_and_trace)
    return res.results


def kernel(x, Wq, Wk, Wv, Wo, bo, W1, b1, W2, b2, g1, g2):
    f32 = lambda a: np.ascontiguousarray(np.asarray(a), dtype=np.float32)
    x = f32(x)
    Wq, Wk, Wv, Wo, bo = f32(Wq), f32(Wk), f32(Wv), f32(Wo), f32(bo)
    W1, b1, W2, b2, g1, g2 = f32(W1), f32(b1), f32(W2), f32(b2), f32(g1), f32(g2)

    B, T, C = x.shape
    H, _, DH = Wq.shape
    HP = H // N_CORES           # heads per core (2)
    DA = DH + 1
    LAST_EXEC_NS.clear()
    LAST_TRACES.clear()

    # ---- phase 1
    nc1 = _phase1(B, T, C, DH)
    xT = np.ascontiguousarray(x.transpose(0, 2, 1)).astype(BF_NP)
    in1 = []
    for i in range(N_CORES):
        pq = Wq[HP * i:HP * (i + 1)].transpose(1, 0, 2).reshape(C, HP * DH)
        pk = Wk[HP * i:HP * (i + 1)].transpose(1, 0, 2).reshape(C, HP * DH)
        pv = Wv[HP * i:HP * (i + 1)].transpose(1, 0, 2).reshape(C, HP * DH)
        in1.append({"xT": xT,
                    "wq": np.ascontiguousarray(pq).astype(BF_NP),
                    "wk": np.ascontiguousarray(pk).astype(BF_NP),
                    "wv": np.ascontiguousarray(pv).astype(BF_NP)})
    res1 = _run(nc1, in1)

    attn = np.empty((B, T, C), np.float32)
    for i in range(N_CORES):
        ot = res1[i]["ot"].astype(np.float32)  # [B, HP, DA, T]
        o = ot[:, :, :DH, :]
        den = ot[:, :, DH, :]
        on = o / den[:, :, None, :]
        for hh in range(HP):
            hcol = (HP * i + hh) * DH
            attn[:, :, hcol:hcol + DH] = on[:, hh].transpose(0, 2, 1)

    # ---- phase 2
    NTOK = B * T // N_CORES
    nc2 = _phase2(NTOK, C, W1.shape[1])
    xf = (x.reshape(B * T, C) + bo).astype(BF_NP)  # fold bo into the residual
    af = attn.reshape(B * T, C)
    wo_bf = Wo.astype(BF_NP)
    w1_bf = W1.astype(BF_NP)
    w2_bf = W2.astype(BF_NP)
    in2 = []
    for k in range(N_CORES):
        sl = slice(k * NTOK, (k + 1) * NTOK)
        in2.append({
            "xc": np.ascontiguousarray(xf[sl]),
            "attnT": np.ascontiguousarray(af[sl].T).astype(BF_NP),
            "wo": wo_bf, "w1": w1_bf, "w2": w2_bf,
            "g1": g1, "g2": g2, "b1": b1, "b2": b2.astype(BF_NP),
        })
    res2 = _run(nc2, in2)
    out = np.concatenate([res2[k]["out"] for k in range(N_CORES)], axis=0)
    return out.reshape(B, T, C)


# revision 22
# speedup vs baseline: 1.1299x; 1.0323x over previous
"""Trainium2 Bass kernel for a dense transformer block, distributed over 8
NeuronCores.

Sharding:
  phase 1 (attention): tensor-parallel over heads — each core computes 2 of
    the 16 heads end-to-end (QKV projections + causal softmax(QK^T)V), and
    returns the unnormalized per-head output O^T together with the softmax
    denominators (obtained via a ones-column appended to V).
  phase 2 (Wo + norms + FFN): data-parallel over tokens — each core handles
    512 of the 4096 token rows with replicated weights.

v2 notes (vs the first working version):
  - QKV projections pipeline against the x DMA stream (c-chunk outer loop).
  - Stripe phase software-pipelines S -> exp -> PV with PV one stripe
    behind, so the PE never waits on the scalar engine's exp.
  - Phase 2 runs Wo token-pair-wise against the weight DMA stream, rmsnorm
    per token block, silu directly on the scalar engine, W2 d-outer with
    residuals injected into PSUM via identity matmuls, per-block output DMA.
  - All matmuls bf16 (inputs quantized host-side); activation tables are
    warmed during initial DMA stalls.
"""

import math
from contextlib import ExitStack

import ml_dtypes
import numpy as np

BF_NP = ml_dtypes.bfloat16

import concourse.bass as bass
import concourse.mybir as mybir
import concourse.tile as tile
from concourse import bacc
from concourse.bass_utils import run_bass_kernel_spmd
from concourse.masks import make_identity, make_upper_triangular

FP = mybir.dt.float32
FPR = mybir.dt.float32r
BF = mybir.dt.bfloat16
AF = mybir.ActivationFunctionType

N_CORES = 8
P = 128
EPS = 1e-6

# exec times (ns) of the most recent kernel() call, one entry per phase, when
# tracing was enabled via BASS_TRACE=1; None entries otherwise.
LAST_EXEC_NS = []
LAST_TRACES = []


# --------------------------------------------------------------------------
# phase 1: per-core attention over a pair of heads
# --------------------------------------------------------------------------

def build_phase1(B, T, C, DH):
    HP = 2                      # heads per core
    DA = DH + 1                 # head dim + ones row (softmax denominator)
    NCC = C // P                # contraction chunks (8)
    NT = T // P                 # 128-token blocks (16)
    NQT = T // 1024             # 1024-wide q tiles per batch (2)
    scale = float(C) ** -0.5    # NOTE: reference scales by C**-0.5, not DH

    nc = bacc.Bacc("TRN2", debug=False)
    xT_d = nc.dram_tensor("xT", [B, C, T], BF, kind="ExternalInput").ap()
    wq_d = nc.dram_tensor("wq", [C, HP * DH], BF, kind="ExternalInput").ap()
    wk_d = nc.dram_tensor("wk", [C, HP * DH], BF, kind="ExternalInput").ap()
    wv_d = nc.dram_tensor("wv", [C, HP * DH], BF, kind="ExternalInput").ap()
    ot_d = nc.dram_tensor("ot", [B, HP, DA, T], BF, kind="ExternalOutput").ap()

    with tile.TileContext(nc) as tc, ExitStack() as ctx:
        const = ctx.enter_context(tc.tile_pool(name="const", bufs=1))
        xpool = ctx.enter_context(tc.tile_pool(name="xp", bufs=1))
        wpool = ctx.enter_context(tc.tile_pool(name="wp", bufs=1))
        qk_pool = ctx.enter_context(tc.tile_pool(name="qk", bufs=1))
        vaug_pool = ctx.enter_context(tc.tile_pool(name="vaug", bufs=1))
        vt_pool = ctx.enter_context(tc.tile_pool(name="vtp", bufs=2))
        pt_pool = ctx.enter_context(tc.tile_pool(name="pt", bufs=3))
        ot_pool = ctx.enter_context(tc.tile_pool(name="otp", bufs=4))

        # weights first (small), so QKV can start on the first x chunk;
        # one consolidated DMA per weight (DGE config time is ~650ns/DMA)
        DW = HP * DH
        wtiles = {}
        for name, src in (("q", wq_d), ("k", wk_d), ("v", wv_d)):
            wt = wpool.tile([P, NCC, DW], BF, tag=f"w{name}", name=f"w{name}")
            nc.sync.dma_start(
                out=wt[:],
                in_=bass.AP(tensor=src.tensor, offset=src.offset,
                            ap=[[DW, P], [P * DW, NCC], [1, DW]]))
            wtiles[name] = wt

        def wchunk(name, c):
            return wtiles[name][:, c, :]

        # x for both batches, 2-chunk granularity so compute chases the DMAs
        xtiles = [[None] * (NCC // 2) for _ in range(B)]
        for b in range(B):
            for cp in range(NCC // 2):
                t = xpool.tile([P, 2, T], BF, tag=f"x{b}_{cp}")
                nc.sync.dma_start(
                    out=t[:],
                    in_=bass.AP(tensor=xT_d.tensor,
                                offset=xT_d.offset + (b * C + cp * 2 * P) * T,
                                ap=[[T, P], [P * T, 2], [1, T]]))
                xtiles[b][cp] = t

        def xchunk(b, c, lo, hi):
            return xtiles[b][c // 2][:, c % 2, lo:hi]

        # warm the exp activation table during the DMA stall
        junk = const.tile([P, 1], FP)
        nc.vector.memset(junk[:], 0.0)
        junk2 = const.tile([P, 1], FP)
        nc.scalar.activation(junk2[:], junk[:], AF.Exp)

        # additive mask for the diagonal 128x128 block of S^T [s', q']:
        # 0 where q' >= s' (causal-valid), -1e30 where q' < s'
        negmask = const.tile([P, P], FP)
        nc.gpsimd.memset(negmask[:], 0.0)
        nc.gpsimd.affine_select(
            out=negmask[:], in_=negmask[:],
            compare_op=mybir.AluOpType.is_ge, fill=-1e30,
            base=0, pattern=[[1, P]], channel_multiplier=-1)
        ident = const.tile([P, P], BF)
        make_identity(nc, ident[:])
        ones_col = const.tile([P, NT * HP, 1], FP)
        nc.vector.memset(ones_col[:], 1.0)

        for b in range(B):
            qt = qk_pool.tile([P, T], BF, tag=f"qt{b}")
            kt = qk_pool.tile([P, T], BF, tag=f"kt{b}")
            vaug = vaug_pool.tile([P, NT * HP, DA], BF, tag=f"vaug{b}")
            nc.vector.tensor_copy(vaug[:, :, DA - 1:DA], ones_col[:])

            # ---- stage A: QKV projections (q/k c-outer against DMA stream)
            with tc.tile_pool(name="qk_ps", bufs=1, space="PSUM") as qk_ps:
                q_ps = qk_ps.tile([P, T], FP, tag="qps", name="qps")
                k_ps = qk_ps.tile([P, T], FP, tag="kps", name="kps")
                for c in range(NCC):
                    for wname, ps in (("q", q_ps), ("k", k_ps)):
                        for n in range(T // 512):
                            nc.tensor.matmul(
                                ps[:, n * 512:(n + 1) * 512],
                                wchunk(wname, c),
                                xchunk(b, c, n * 512, (n + 1) * 512),
                                start=(c == 0), stop=(c == NCC - 1))
                for half in range(T // 1024):
                    sl = slice(half * 1024, (half + 1) * 1024)
                    nc.vector.tensor_copy(qt[:, sl], q_ps[:, sl])
                    nc.vector.tensor_copy(kt[:, sl], k_ps[:, sl])

            # V projection + transpose into [s, d] layout with ones column
            with tc.tile_pool(name="v_ps", bufs=2, space="PSUM") as v_ps, \
                 tc.tile_pool(name="tp_ps", bufs=2, space="PSUM") as tp_ps:
                for n8 in range(T // 512):
                    vps = v_ps.tile([P, 512], FP, tag="v")
                    for c in range(NCC):
                        nc.tensor.matmul(
                            vps[:], wchunk("v", c),
                            xchunk(b, c, n8 * 512, (n8 + 1) * 512),
                            start=(c == 0), stop=(c == NCC - 1))
                    vt = vt_pool.tile([P, 512], BF, tag="vt")
                    nc.vector.tensor_copy(vt[:], vps[:])
                    tp = tp_ps.tile([P, 4, P], BF, tag="tp")
                    for u in range(4):
                        nc.tensor.transpose(
                            tp[:, u, :], vt[:, u * P:(u + 1) * P], ident[:])
                    # [tok, (j,h)-dh] -> vaug slots 8*n8 .. 8*n8+8 (64 of 65)
                    nc.vector.tensor_copy(
                        vaug[:, n8 * 8:(n8 + 1) * 8, 0:DH], tp[:])

            # ---- stage B: causal stripes, PV pipelined one stripe behind
            with tc.tile_pool(name="s_ps", bufs=1, space="PSUM") as s_ps, \
                 tc.tile_pool(name="o_ps", bufs=1, space="PSUM") as o_ps:
                for kq in range(NQT):
                    q_lo = 1024 * kq
                    q_hi = 1024 * (kq + 1)
                    o_t = [o_ps.tile([DA, 1024], FP, tag=f"o{h}", name=f"o{h}")
                           for h in range(HP)]
                    pend = []

                    def emit_pv(ent):
                        j, a0, chunks, ptks = ent
                        for h in range(HP):
                            va = vaug[:, j * HP + h, :]
                            for (a, e) in chunks:
                                last_j = e // P - 1
                                nc.tensor.matmul(
                                    o_t[h][:, a - q_lo:e - q_lo],
                                    va, ptks[h][:, a - q_lo:e - q_lo],
                                    start=(j == 0), stop=(j == last_j))

                    for j in range(8 * (kq + 1)):
                        s0 = j * P
                        a0 = max(s0, q_lo)
                        chunks = []
                        for m in range(a0 // 512, q_hi // 512):
                            a = max(a0, m * 512)
                            chunks.append((a, (m + 1) * 512))
                        stl = [s_ps.tile([P, 1024], FP, tag=f"s{h}", name=f"s{h}")
                               for h in range(HP)]
                        for (a, e) in chunks:
                            for h in range(HP):
                                hs = slice(h * DH, (h + 1) * DH)
                                nc.tensor.matmul(
                                    stl[h][:, a - q_lo:e - q_lo],
                                    kt[hs, s0:s0 + P], qt[hs, a:e],
                                    start=True, stop=True,
                                    tile_position=(h * DH, 0))
                        if q_lo <= s0:
                            for h in range(HP):
                                nc.vector.tensor_add(
                                    stl[h][:, s0 - q_lo:s0 - q_lo + P],
                                    stl[h][:, s0 - q_lo:s0 - q_lo + P],
                                    negmask[:])
                        ptks = []
                        for h in range(HP):
                            ptk = pt_pool.tile([P, 1024], BF, tag=f"pt{h}")
                            nc.scalar.activation(
                                ptk[:, a0 - q_lo:1024], stl[h][:, a0 - q_lo:1024],
                                AF.Exp, scale=scale)
                            ptks.append(ptk)
                        pend.append((j, a0, chunks, ptks))
                        if len(pend) >= 2:
                            emit_pv(pend.pop(0))
                    emit_pv(pend.pop(0))
                    for h in range(HP):
                        osb = ot_pool.tile([DA, 1024], BF, tag=f"ot{h}")
                        nc.vector.tensor_copy(osb[:], o_t[h][:])
                        nc.gpsimd.dma_start(
                            out=ot_d[b, h, :, q_lo:q_hi], in_=osb[:])
    nc.compile()
    return nc


# --------------------------------------------------------------------------
# phase 2: per-core Wo projection + residual + rmsnorm + FFN + rmsnorm
# --------------------------------------------------------------------------

def build_phase2(NTOK, C, DFF):
    NTB = NTOK // P             # 4 token blocks
    NCH = C // P                # 8 channel chunks
    NDF = DFF // P              # 32 dff chunks
    NG = DFF // 512             # 8 w1 groups

    nc = bacc.Bacc("TRN2", debug=False)
    xc_d = nc.dram_tensor("xc", [NTOK, C], BF, kind="ExternalInput").ap()
    at_d = nc.dram_tensor("attnT", [C, NTOK], BF, kind="ExternalInput").ap()
    wo_d = nc.dram_tensor("wo", [C, C], BF, kind="ExternalInput").ap()
    w1_d = nc.dram_tensor("w1", [C, DFF], BF, kind="ExternalInput").ap()
    w2_d = nc.dram_tensor("w2", [DFF, C], BF, kind="ExternalInput").ap()
    g1_d = nc.dram_tensor("g1", [C], FP, kind="ExternalInput").ap()
    g2_d = nc.dram_tensor("g2", [C], FP, kind="ExternalInput").ap()
    b1_d = nc.dram_tensor("b1", [DFF], FP, kind="ExternalInput").ap()
    b2_d = nc.dram_tensor("b2", [C], BF, kind="ExternalInput").ap()
    out_d = nc.dram_tensor("out", [NTOK, C], FP, kind="ExternalOutput").ap()

    def bcast_rows(src_ap, cols):
        # DRAM vector [cols] -> [P, cols] (same row in every partition)
        return bass.AP(tensor=src_ap.tensor, offset=src_ap.offset,
                       ap=[[0, P], [1, cols]])

    halves = ((0, 512), (512, 512))

    with tile.TileContext(nc) as tc, ExitStack() as ctx:
        const = ctx.enter_context(tc.tile_pool(name="const", bufs=1))
        work = ctx.enter_context(tc.tile_pool(name="work", bufs=2))
        stats = ctx.enter_context(tc.tile_pool(name="stats", bufs=4))
        h_pool = ctx.enter_context(tc.tile_pool(name="hp", bufs=1))
        at_pool = ctx.enter_context(tc.tile_pool(name="atp", bufs=1))
        out_pool = ctx.enter_context(tc.tile_pool(name="outp", bufs=2))

        # --- input DMAs, consolidated (DGE config is ~650ns per DMA) and in
        # consumption order (sync queue; outputs go on the gpsimd queue so
        # they never sit behind these)
        wo_po = ctx.enter_context(tc.tile_pool(name="wop", bufs=1))
        att_t, wot_t = [], []
        for g in range(2):
            at4 = wo_po.tile([P, 4, NTOK], BF, tag=f"at{g}", name=f"at{g}")
            nc.sync.dma_start(
                out=at4[:],
                in_=bass.AP(tensor=at_d.tensor,
                            offset=at_d.offset + g * 4 * P * NTOK,
                            ap=[[NTOK, P], [P * NTOK, 4], [1, NTOK]]))
            att_t.append(at4)
            wo4 = wo_po.tile([P, 4, C], BF, tag=f"wo{g}", name=f"wo{g}")
            nc.sync.dma_start(
                out=wo4[:],
                in_=bass.AP(tensor=wo_d.tensor,
                            offset=wo_d.offset + g * 4 * P * C,
                            ap=[[C, P], [P * C, 4], [1, C]]))
            wot_t.append(wo4)

        def attc(c, lo, hi):
            return att_t[c // 4][:, c % 4, lo:hi]

        def woc(c, lo, hi):
            return wot_t[c // 4][:, c % 4, lo:hi]

        xc_t = wo_po.tile([P, NTB, C], BF, tag="xc", name="xc")
        nc.sync.dma_start(
            out=xc_t[:],
            in_=bass.AP(tensor=xc_d.tensor, offset=xc_d.offset,
                        ap=[[C, P], [P * C, NTB], [1, C]]))
        eps_t = const.tile([P, 1], FP)
        nc.vector.memset(eps_t[:], EPS)
        g1b = const.tile([P, C], FP)
        nc.sync.dma_start(out=g1b[:], in_=bcast_rows(g1_d, C))
        g2b = const.tile([P, C], FP)
        nc.sync.dma_start(out=g2b[:], in_=bcast_rows(g2_d, C))
        b2row = const.tile([1, C], BF)
        nc.sync.dma_start(
            out=b2row[:],
            in_=bass.AP(tensor=b2_d.tensor, offset=b2_d.offset,
                        ap=[[0, 1], [1, C]]))
        ones1 = const.tile([1, P], BF)
        nc.vector.memset(ones1[:], 1.0)
        # b1 as [P, NDF]: element (p, d) = b1[d*P + p]
        b1s = const.tile([P, NDF], FP)
        nc.sync.dma_start(
            out=b1s[:],
            in_=bass.AP(tensor=b1_d.tensor, offset=b1_d.offset,
                        ap=[[1, P], [P, NDF]]))
        # w1: one DMA per 512-wide group, streamed via a 3-deep window
        w1_po = ctx.enter_context(tc.tile_pool(name="w1p", bufs=3))
        w1g = []
        for g in range(NG):
            t = w1_po.tile([P, NCH, 512], BF, tag="w1g", name="w1g")
            nc.sync.dma_start(
                out=t[:],
                in_=bass.AP(tensor=w1_d.tensor,
                            offset=w1_d.offset + g * 512,
                            ap=[[DFF, P], [P * DFF, NCH], [1, 512]]))
            w1g.append(t)
        # w2: one DMA per 4 d-chunks, streamed via a 3-deep window
        w2_po = ctx.enter_context(tc.tile_pool(name="w2p", bufs=3))
        w2q = []
        for q in range(NDF // 4):
            t = w2_po.tile([P, 4, C], BF, tag="w2q", name="w2q")
            nc.sync.dma_start(
                out=t[:],
                in_=bass.AP(tensor=w2_d.tensor,
                            offset=w2_d.offset + q * 4 * P * C,
                            ap=[[C, P], [P * C, 4], [1, C]]))
            w2q.append(t)

        def w2c(d, lo, hi):
            return w2q[d // 4][:, d % 4, lo:hi]

        # warm the sqrt table during the DMA stall
        junk = const.tile([P, 1], FP)
        nc.vector.memset(junk[:], 1.0)
        junk2 = const.tile([P, 1], FP)
        nc.scalar.activation(junk2[:], junk[:], AF.Sqrt)
        ident = const.tile([P, P], BF)
        make_identity(nc, ident[:])

        hT = h_pool.tile([P, NCH, NTOK], BF, tag="hT")      # h^T, c-major
        h_bfs = []

        # ---- stage 0: o = attn@Wo + x (residual via identity matmul into
        # PSUM); rmsnorm per token block; transpose into hT
        with tc.tile_pool(name="o_ps", bufs=1, space="PSUM") as o_ps, \
             tc.tile_pool(name="t_ps", bufs=2, space="PSUM") as t_ps:
            for pair_i in range(NTB // 2):
                pair = (2 * pair_i, 2 * pair_i + 1)
                o2 = [[o_ps.tile([P, 512], FP, tag=f"o2_{si}_{hi}",
                                 name=f"o2_{si}_{hi}")
                       for hi in range(2)] for si in range(2)]
                for c in range(NCH):
                    for si, tb in enumerate(pair):
                        for hi, (hst, hw) in enumerate(halves):
                            nc.tensor.matmul(
                                o2[si][hi][:],
                                attc(c, tb * P, (tb + 1) * P),
                                woc(c, hst, hst + hw),
                                start=(c == 0), stop=False)
                for si, tb in enumerate(pair):
                    for hi, (hst, hw) in enumerate(halves):
                        nc.tensor.matmul(
                            o2[si][hi][:], ident[:],
                            xc_t[:, tb, hst:hst + hw],
                            start=False, stop=True)
                for si, tb in enumerate(pair):
                    sq = work.tile([P, 512], FP, tag="sq")
                    ss = [stats.tile([P, 1], FP, tag=f"ss{hi}", name=f"ss{hi}")
                          for hi in range(2)]
                    for hi in range(2):
                        nc.scalar.activation(sq[:], o2[si][hi][:], AF.Square,
                                             accum_out=ss[hi][:])
                    nc.vector.tensor_add(ss[0][:], ss[0][:], ss[1][:])
                    rstd = stats.tile([P, 1], FP, tag="rstd")
                    nc.scalar.activation(rstd[:], ss[0][:], AF.Sqrt,
                                         scale=1.0 / C, bias=eps_t[:])
                    rinv = stats.tile([P, 1], FP, tag="rinv")
                    nc.vector.reciprocal(rinv[:], rstd[:])
                    h_bf = h_pool.tile([P, C], BF, tag=f"h{tb}")
                    for hi, (hst, hw) in enumerate(halves):
                        nc.vector.scalar_tensor_tensor(
                            h_bf[:, hst:hst + hw], o2[si][hi][:], rinv[:],
                            g1b[:, hst:hst + hw],
                            op0=mybir.AluOpType.mult, op1=mybir.AluOpType.mult)
                    h_bfs.append(h_bf)
                    for g4 in range(2):
                        tp = t_ps.tile([P, 4, P], BF, tag="tp")
                        for u in range(4):
                            nc.tensor.transpose(
                                tp[:, u, :],
                                h_bf[:, (g4 * 4 + u) * P:(g4 * 4 + u + 1) * P],
                                ident[:])
                        nc.vector.tensor_copy(
                            hT[:, g4 * 4:(g4 + 1) * 4, tb * P:(tb + 1) * P],
                            tp[:])

        # ---- stage 1: aT = silu(W1^T @ h^T + b1), directly via scalar silu
        ats = []
        with tc.tile_pool(name="a_ps", bufs=2, space="PSUM") as a_ps:
            for g in range(NG):
                aps = [a_ps.tile([P, NTOK], FP, tag=f"a{u}", name=f"a{u}")
                       for u in range(4)]
                for c in range(NCH):
                    for u in range(4):
                        nc.tensor.matmul(
                            aps[u][:], w1g[g][:, c, u * P:(u + 1) * P],
                            hT[:, c, :],
                            start=(c == 0), stop=(c == NCH - 1))
                for u in range(4):
                    d = 4 * g + u
                    at_t = at_pool.tile([P, NTOK], BF, tag=f"at{d}")
                    nc.scalar.activation(at_t[:], aps[u][:], AF.Silu,
                                         bias=b1s[:, d:d + 1], scale=1.0)
                    ats.append(at_t)

        # ---- stage 2: f = aT^T @ W2 + h + b2 (residuals via identity/ones
        # matmuls); d-outer so w2 streams; rmsnorm + store per token block
        with tc.tile_pool(name="f_ps", bufs=1, space="PSUM") as f_ps:
            f2 = [f_ps.tile([P, C], FP, tag=f"f{tb}", name=f"f{tb}")
                  for tb in range(NTB)]
            for tb in range(NTB):
                for (hst, hw) in halves:
                    nc.tensor.matmul(
                        f2[tb][:, hst:hst + hw], ident[:],
                        h_bfs[tb][:, hst:hst + hw],
                        start=True, stop=False)
                    nc.tensor.matmul(
                        f2[tb][:, hst:hst + hw], ones1[:],
                        b2row[:, hst:hst + hw],
                        start=False, stop=False)
            for d in range(NDF - 4):
                for tb in range(NTB):
                    for (hst, hw) in halves:
                        nc.tensor.matmul(
                            f2[tb][:, hst:hst + hw],
                            ats[d][:, tb * P:(tb + 1) * P],
                            w2c(d, hst, hst + hw),
                            start=False, stop=False)
            # stagger the last 4 d-chunks per token block so outputs finish
            # (and norm+store) incrementally instead of all at once
            for tb in range(NTB):
                for d in range(NDF - 4, NDF):
                    for (hst, hw) in halves:
                        nc.tensor.matmul(
                            f2[tb][:, hst:hst + hw],
                            ats[d][:, tb * P:(tb + 1) * P],
                            w2c(d, hst, hst + hw),
                            start=False, stop=(d == NDF - 1))
                sq = work.tile([P, C], FP, tag="sq2")
                ssum = stats.tile([P, 1], FP, tag="ssum2")
                nc.scalar.activation(sq[:], f2[tb][:], AF.Square,
                                     accum_out=ssum[:])
                rstd = stats.tile([P, 1], FP, tag="rstd2")
                nc.scalar.activation(rstd[:], ssum[:], AF.Sqrt,
                                     scale=1.0 / C, bias=eps_t[:])
                rinv = stats.tile([P, 1], FP, tag="rinv2")
                nc.vector.reciprocal(rinv[:], rstd[:])
                o = out_pool.tile([P, C], FP, tag="outt")
                nc.vector.scalar_tensor_tensor(
                    o[:], f2[tb][:], rinv[:], g2b[:],
                    op0=mybir.AluOpType.mult, op1=mybir.AluOpType.mult)
                nc.gpsimd.dma_start(
                    out=out_d[tb * P:(tb + 1) * P, :], in_=o[:])
    nc.compile()
    return nc


# --------------------------------------------------------------------------
# host orchestration
# --------------------------------------------------------------------------

_CACHE = {}


def _phase1(B, T, C, DH):
    key = ("p1", B, T, C, DH)
    if key not in _CACHE:
        _CACHE[key] = build_phase1(B, T, C, DH)
    return _CACHE[key]


def _phase2(NTOK, C, DFF):
    key = ("p2", NTOK, C, DFF)
    if key not in _CACHE:
        _CACHE[key] = build_phase2(NTOK, C, DFF)
    return _CACHE[key]


def _run(nc, in_maps):
    import os
    trace = bool(os.environ.get("KERNEL_TRACE"))
    res = run_bass_kernel_spmd(nc, in_maps, core_ids=list(range(N_CORES)),
                               trace=trace)
    LAST_EXEC_NS.append(res.exec_time_ns)
    LAST_TRACES.append(res.instructions_and_trace)
    return res.results


def kernel(x, Wq, Wk, Wv, Wo, bo, W1, b1, W2, b2, g1, g2):
    f32 = lambda a: np.ascontiguousarray(np.asarray(a), dtype=np.float32)
    x = f32(x)
    Wq, Wk, Wv, Wo, bo = f32(Wq), f32(Wk), f32(Wv), f32(Wo), f32(bo)
    W1, b1, W2, b2, g1, g2 = f32(W1), f32(b1), f32(W2), f32(b2), f32(g1), f32(g2)

    B, T, C = x.shape
    H, _, DH = Wq.shape
    HP = H // N_CORES           # heads per core (2)
    DA = DH + 1
    LAST_EXEC_NS.clear()
    LAST_TRACES.clear()

    # ---- phase 1
    nc1 = _phase1(B, T, C, DH)
    xT = np.ascontiguousarray(x.transpose(0, 2, 1)).astype(BF_NP)
    in1 = []
    for i in range(N_CORES):
        pq = Wq[HP * i:HP * (i + 1)].transpose(1, 0, 2).reshape(C, HP * DH)
        pk = Wk[HP * i:HP * (i + 1)].transpose(1, 0, 2).reshape(C, HP * DH)
        pv = Wv[HP * i:HP * (i + 1)].transpose(1, 0, 2).reshape(C, HP * DH)
        in1.append({"xT": xT,
                    "wq": np.ascontiguousarray(pq).astype(BF_NP),
                    "wk": np.ascontiguousarray(pk).astype(BF_NP),
                    "wv": np.ascontiguousarray(pv).astype(BF_NP)})
    res1 = _run(nc1, in1)

    attn = np.empty((B, T, C), np.float32)
    for i in range(N_CORES):
        ot = res1[i]["ot"].astype(np.float32)  # [B, HP, DA, T]
        o = ot[:, :, :DH, :]
        den = ot[:, :, DH, :]
        on = o / den[:, :, None, :]
        for hh in range(HP):
            hcol = (HP * i + hh) * DH
            attn[:, :, hcol:hcol + DH] = on[:, hh].transpose(0, 2, 1)

    # ---- phase 2
    NTOK = B * T // N_CORES
    nc2 = _phase2(NTOK, C, W1.shape[1])
    xf = (x.reshape(B * T, C) + bo).astype(BF_NP)  # fold bo into the residual
    af = attn.reshape(B * T, C)
    wo_bf = Wo.astype(BF_NP)
    w1_bf = W1.astype(BF_NP)
    w2_bf = W2.astype(BF_NP)
    in2 = []
    for k in range(N_CORES):
        sl = slice(k * NTOK, (k + 1) * NTOK)
        in2.append({
            "xc": np.ascontiguousarray(xf[sl]),
            "attnT": np.ascontiguousarray(af[sl].T).astype(BF_NP),
            "wo": wo_bf, "w1": w1_bf, "w2": w2_bf,
            "g1": g1, "g2": g2, "b1": b1, "b2": b2.astype(BF_NP),
        })
    res2 = _run(nc2, in2)
    out = np.concatenate([res2[k]["out"] for k in range(N_CORES)], axis=0)
    return out.reshape(B, T, C)


# revision 26
# speedup vs baseline: 1.2028x; 1.0645x over previous
"""Trainium2 Bass kernel for a dense transformer block, distributed over 8
NeuronCores.

Sharding:
  phase 1 (attention): tensor-parallel over heads — each core computes 2 of
    the 16 heads end-to-end (QKV projections + causal softmax(QK^T)V), and
    returns the unnormalized per-head output O^T together with the softmax
    denominators (obtained via a ones-column appended to V).
  phase 2 (Wo + norms + FFN): data-parallel over tokens — each core handles
    512 of the 4096 token rows with replicated weights.

v4 notes:
  - All DRAM inputs are host-packed so every DMA reads a contiguous region
    (>=2KB per partition line); DMAs are spread over the sync/scalar/vector
    queues to use multiple DMA rings in parallel.
  - Phase 1 interleaves batch-1 QKV work as PE filler into batch-0's
    exp-bound stripe phase (keeps the PE HAM clock-gate warm); batch 1's
    stripe phase merges both heads into one PSUM tile with a single wide
    exp per stripe to cut scalar-engine overhead.
  - Phase 2: Wo stage streams against the DMA with 3 rotating PSUM slots;
    residuals are injected into PSUM via identity/ones matmuls; silu runs
    directly on the scalar engine; the last 8 W2 d-chunks are staggered per
    token block so outputs drain incrementally; outputs are bf16.
"""

import math
from contextlib import ExitStack

import ml_dtypes
import numpy as np

BF_NP = ml_dtypes.bfloat16

import concourse.bass as bass
import concourse.mybir as mybir
import concourse.tile as tile
from concourse import bacc
from concourse.bass_utils import run_bass_kernel_spmd
from concourse.masks import make_identity

FP = mybir.dt.float32
BF = mybir.dt.bfloat16
AF = mybir.ActivationFunctionType

N_CORES = 8
P = 128
EPS = 1e-6

LAST_EXEC_NS = []
LAST_TRACES = []


# --------------------------------------------------------------------------
# phase 1: per-core attention over a pair of heads
# --------------------------------------------------------------------------

def build_phase1(B, T, C, DH):
    HP = 2                      # heads per core
    DA = DH + 1                 # head dim + ones row (softmax denominator)
    NCC = C // P                # contraction chunks (8)
    NT = T // P                 # 128-token blocks (16)
    NQT = T // 1024             # 1024-wide q tiles per batch (2)
    DW = HP * DH                # packed head dims (128)
    scale = float(C) ** -0.5    # NOTE: reference scales by C**-0.5, not DH

    nc = bacc.Bacc("TRN2", debug=False)
    # host-packed layouts (contiguous per DMA)
    xT_d = nc.dram_tensor("xT", [B, NCC // 2, P, 2 * T], BF,
                          kind="ExternalInput").ap()
    wq_d = nc.dram_tensor("wq", [P, NCC * DW], BF, kind="ExternalInput").ap()
    wk_d = nc.dram_tensor("wk", [P, NCC * DW], BF, kind="ExternalInput").ap()
    wv_d = nc.dram_tensor("wv", [P, NCC * DW], BF, kind="ExternalInput").ap()
    ot_d = nc.dram_tensor("ot", [B, HP, DA, T], BF, kind="ExternalOutput").ap()

    with tile.TileContext(nc) as tc, ExitStack() as ctx:
        const = ctx.enter_context(tc.tile_pool(name="const", bufs=1))
        xpool = ctx.enter_context(tc.tile_pool(name="xp", bufs=1))
        wpool = ctx.enter_context(tc.tile_pool(name="wp", bufs=1))
        qk_pool = ctx.enter_context(tc.tile_pool(name="qk", bufs=1))
        vaug_pool = ctx.enter_context(tc.tile_pool(name="vaug", bufs=1))
        vt_pool = ctx.enter_context(tc.tile_pool(name="vtp", bufs=2))
        pt_pool = ctx.enter_context(tc.tile_pool(name="pt", bufs=3))
        ot_pool = ctx.enter_context(tc.tile_pool(name="otp", bufs=2))

        # weights + x DMAs, spread across queues, in consumption order
        wq_t = wpool.tile([P, NCC * DW], BF, tag="wq")
        nc.sync.dma_start(out=wq_t[:], in_=wq_d)
        wk_t = wpool.tile([P, NCC * DW], BF, tag="wk")
        nc.scalar.dma_start(out=wk_t[:], in_=wk_d)
        xts = [[None] * (NCC // 2) for _ in range(B)]
        for b in range(B):
            for cp in range(NCC // 2):
                t = xpool.tile([P, 2 * T], BF, tag=f"x{b}_{cp}")
                eng = nc.sync if cp % 2 == 0 else nc.scalar
                eng.dma_start(out=t[:], in_=xT_d[b, cp])
                xts[b][cp] = t
        wv_t = wpool.tile([P, NCC * DW], BF, tag="wv")
        nc.gpsimd.dma_start(out=wv_t[:], in_=wv_d)
        wtile = {"q": wq_t, "k": wk_t, "v": wv_t}

        def wchunk(name, c):
            return wtile[name][:, c * DW:(c + 1) * DW]

        def xchunk(b, c, lo, hi):
            half = (c % 2) * T
            return xts[b][c // 2][:, half + lo:half + hi]

        # warm the exp table during the DMA stall
        junk = const.tile([P, 1], FP)
        nc.vector.memset(junk[:], 0.0)
        junk2 = const.tile([P, 1], FP)
        nc.scalar.activation(junk2[:], junk[:], AF.Exp)

        negmask = const.tile([P, P], FP)
        nc.gpsimd.memset(negmask[:], 0.0)
        nc.gpsimd.affine_select(
            out=negmask[:], in_=negmask[:],
            compare_op=mybir.AluOpType.is_ge, fill=-1e30,
            base=0, pattern=[[1, P]], channel_multiplier=-1)
        ident = const.tile([P, P], BF)
        make_identity(nc, ident[:])
        ones_col = const.tile([P, NT * HP, 1], FP)
        nc.vector.memset(ones_col[:], 1.0)

        qts, kts, vaugs = [], [], []
        for b in range(B):
            qts.append(qk_pool.tile([P, T], BF, tag=f"qt{b}", name=f"qt{b}"))
            kts.append(qk_pool.tile([P, T], BF, tag=f"kt{b}", name=f"kt{b}"))
            va = vaug_pool.tile([P, NT * HP, DA], BF, tag=f"va{b}",
                                name=f"va{b}")
            nc.vector.tensor_copy(va[:, :, DA - 1:DA], ones_col[:])
            vaugs.append(va)

        # ---- stage A(b0): q/k projections, c-outer against the DMA stream
        b0 = 0
        with tc.tile_pool(name="qk_ps", bufs=1, space="PSUM") as qk_ps:
            q_ps = qk_ps.tile([P, T], FP, tag="qps", name="qps")
            k_ps = qk_ps.tile([P, T], FP, tag="kps", name="kps")
            for c in range(NCC):
                for wname, ps in (("q", q_ps), ("k", k_ps)):
                    for n in range(T // 512):
                        nc.tensor.matmul(
                            ps[:, n * 512:(n + 1) * 512],
                            wchunk(wname, c),
                            xchunk(b0, c, n * 512, (n + 1) * 512),
                            start=(c == 0), stop=(c == NCC - 1))
            for half in range(T // 1024):
                sl = slice(half * 1024, (half + 1) * 1024)
                nc.vector.tensor_copy(qts[b0][:, sl], q_ps[:, sl])
                nc.vector.tensor_copy(kts[b0][:, sl], k_ps[:, sl])

        # ---- filler quanta (run inside b0's stripe sections) + b0 V head
        fl_ctx = ExitStack()
        fl_ps = fl_ctx.enter_context(
            tc.tile_pool(name="fl_ps", bufs=2, space="PSUM"))

        def v_quant(b, n8):
            vps = fl_ps.tile([P, 512], FP, tag="proj", name="proj")
            for c in range(NCC):
                nc.tensor.matmul(
                    vps[:], wchunk("v", c),
                    xchunk(b, c, n8 * 512, (n8 + 1) * 512),
                    start=(c == 0), stop=(c == NCC - 1))
            vt = vt_pool.tile([P, 512], BF, tag="vt")
            nc.vector.tensor_copy(vt[:], vps[:])
            tp = fl_ps.tile([P, 4, P], BF, tag="tp", name="tp")
            for u in range(4):
                nc.tensor.transpose(
                    tp[:, u, :], vt[:, u * P:(u + 1) * P], ident[:])
            nc.vector.tensor_copy(
                vaugs[b][:, n8 * 8:(n8 + 1) * 8, 0:DH], tp[:])

        def qk_quant(b, name, n8):
            dst = qts[b] if name == "q" else kts[b]
            ps = fl_ps.tile([P, 512], FP, tag="proj", name="proj")
            for c in range(NCC):
                nc.tensor.matmul(
                    ps[:], wchunk(name, c),
                    xchunk(b, c, n8 * 512, (n8 + 1) * 512),
                    start=(c == 0), stop=(c == NCC - 1))
            nc.vector.tensor_copy(dst[:, n8 * 512:(n8 + 1) * 512], ps[:])

        # b0 needs V n0/n1 before its first PV; emit them up front
        v_quant(b0, 0)
        v_quant(b0, 1)
        b1 = 1
        filler = [lambda: v_quant(b0, 2), lambda: v_quant(b0, 3)]
        for n8 in (0, 1):
            filler.append(lambda n=n8: qk_quant(b1, "k", n))
        for n8 in (0, 1):
            filler.append(lambda n=n8: qk_quant(b1, "q", n))
        for n8 in (0, 1):
            filler.append(lambda n=n8: v_quant(b1, n))
        for n8 in (2, 3):
            filler.append(lambda n=n8: qk_quant(b1, "k", n))
        for n8 in (2, 3):
            filler.append(lambda n=n8: qk_quant(b1, "q", n))
        for n8 in (2, 3):
            filler.append(lambda n=n8: v_quant(b1, n))

        def chunks_for(a0, q_hi):
            out = []
            for m in range(a0 // 512, q_hi // 512):
                out.append((max(a0, m * 512), (m + 1) * 512))
            return out

        # ---- b0 stripe sections (single head), filler every 2nd stripe
        with tc.tile_pool(name="s_ps", bufs=1, space="PSUM") as s_ps, \
             tc.tile_pool(name="o_ps", bufs=1, space="PSUM") as o_ps:
            for (h, kq) in ((0, 0), (0, 1), (1, 0), (1, 1)):
                q_lo, q_hi = 1024 * kq, 1024 * (kq + 1)
                hs = slice(h * DH, (h + 1) * DH)
                o_t = o_ps.tile([DA, 1024], FP, tag="o", name="o")
                pend = None

                def emit_pv(ent):
                    j, chunks, ptk = ent
                    va = vaugs[b0][:, j * HP + h, :]
                    for (a, e) in chunks:
                        nc.tensor.matmul(
                            o_t[:, a - q_lo:e - q_lo],
                            va, ptk[:, a - q_lo:e - q_lo],
                            start=(j == 0), stop=(j == e // P - 1))

                for j in range(8 * (kq + 1)):
                    s0 = j * P
                    a0 = max(s0, q_lo)
                    chunks = chunks_for(a0, q_hi)
                    stl = s_ps.tile([P, 1024], FP, tag="s", name="s")
                    for (a, e) in chunks:
                        nc.tensor.matmul(
                            stl[:, a - q_lo:e - q_lo],
                            kts[b0][hs, s0:s0 + P], qts[b0][hs, a:e],
                            start=True, stop=True)
                    if q_lo <= s0:
                        nc.vector.tensor_add(
                            stl[:, s0 - q_lo:s0 - q_lo + P],
                            stl[:, s0 - q_lo:s0 - q_lo + P], negmask[:])
                    ptk = pt_pool.tile([P, 1024], BF, tag="pt")
                    nc.scalar.activation(
                        ptk[:, a0 - q_lo:1024], stl[:, a0 - q_lo:1024],
                        AF.Exp, scale=scale)
                    if pend is not None:
                        emit_pv(pend)
                    pend = (j, chunks, ptk)
                    if j % 2 == 0 and filler:
                        filler.pop(0)()
                emit_pv(pend)
                osb = ot_pool.tile([DA, 1024], BF, tag="osb")
                nc.vector.tensor_copy(osb[:], o_t[:])
                nc.gpsimd.dma_start(
                    out=ot_d[b0, h, :, q_lo:q_hi], in_=osb[:])
        while filler:
            filler.pop(0)()
        fl_ctx.close()

        # ---- b1 stripe sections: both heads merged, one wide exp/stripe
        with tc.tile_pool(name="s2_ps", bufs=1, space="PSUM") as s2_ps, \
             tc.tile_pool(name="o2_ps", bufs=1, space="PSUM") as o2_ps:
            for kq in range(NQT):
                q_lo, q_hi = 1024 * kq, 1024 * (kq + 1)
                o_t = [o2_ps.tile([DA, 1024], FP, tag=f"o{h}", name=f"o{h}")
                       for h in range(HP)]
                pend = None

                def emit_pv2(ent):
                    j, chunks, ptk2 = ent
                    for h in range(HP):
                        va = vaugs[b1][:, j * HP + h, :]
                        for (a, e) in chunks:
                            nc.tensor.matmul(
                                o_t[h][:, a - q_lo:e - q_lo],
                                va, ptk2[:, h, a - q_lo:e - q_lo],
                                start=(j == 0), stop=(j == e // P - 1))

                for j in range(8 * (kq + 1)):
                    s0 = j * P
                    a0 = max(s0, q_lo)
                    chunks = chunks_for(a0, q_hi)
                    stl2 = s2_ps.tile([P, HP, 1024], FP, tag="s2", name="s2")
                    for (a, e) in chunks:
                        for h in range(HP):
                            hs = slice(h * DH, (h + 1) * DH)
                            nc.tensor.matmul(
                                stl2[:, h, a - q_lo:e - q_lo],
                                kts[b1][hs, s0:s0 + P], qts[b1][hs, a:e],
                                start=True, stop=True)
                    if q_lo <= s0:
                        for h in range(HP):
                            nc.vector.tensor_add(
                                stl2[:, h, s0 - q_lo:s0 - q_lo + P],
                                stl2[:, h, s0 - q_lo:s0 - q_lo + P],
                                negmask[:])
                    ptk2 = pt_pool.tile([P, HP, 1024], BF, tag="pt2")
                    nc.scalar.activation(
                        ptk2[:, :, a0 - q_lo:1024],
                        stl2[:, :, a0 - q_lo:1024], AF.Exp, scale=scale)
                    if pend is not None:
                        emit_pv2(pend)
                    pend = (j, chunks, ptk2)
                emit_pv2(pend)
                for h in range(HP):
                    osb = ot_pool.tile([DA, 1024], BF, tag="osb2")
                    nc.vector.tensor_copy(osb[:], o_t[h][:])
                    nc.gpsimd.dma_start(
                        out=ot_d[b1, h, :, q_lo:q_hi], in_=osb[:])
    nc.compile()
    return nc


# --------------------------------------------------------------------------
# phase 2: per-core Wo projection + residual + rmsnorm + FFN + rmsnorm
# --------------------------------------------------------------------------

def build_phase2(NTOK, C, DFF):
    NTB = NTOK // P             # 4 token blocks
    NCH = C // P                # 8 channel chunks
    NDF = DFF // P              # 32 dff chunks
    NG = DFF // 512             # 8 w1 groups
    STAG = 8                    # staggered tail d-chunks per token block

    nc = bacc.Bacc("TRN2", debug=False)
    # host-packed layouts (contiguous per DMA)
    xc_d = nc.dram_tensor("xc", [P, NTB * C], BF, kind="ExternalInput").ap()
    at_d = nc.dram_tensor("attnT", [2, P, 4 * NTOK], BF,
                          kind="ExternalInput").ap()
    wo_d = nc.dram_tensor("wo", [2, P, 4 * C], BF, kind="ExternalInput").ap()
    w1_d = nc.dram_tensor("w1", [NG, P, NCH * 512], BF,
                          kind="ExternalInput").ap()
    w2_d = nc.dram_tensor("w2", [NDF // 4, P, 4 * C], BF,
                          kind="ExternalInput").ap()
    g1_d = nc.dram_tensor("g1", [C], FP, kind="ExternalInput").ap()
    g2_d = nc.dram_tensor("g2", [C], FP, kind="ExternalInput").ap()
    b1_d = nc.dram_tensor("b1", [DFF], FP, kind="ExternalInput").ap()
    b2_d = nc.dram_tensor("b2", [C], BF, kind="ExternalInput").ap()
    out_d = nc.dram_tensor("out", [NTOK, C], BF, kind="ExternalOutput").ap()

    def bcast_rows(src_ap, cols):
        return bass.AP(tensor=src_ap.tensor, offset=src_ap.offset,
                       ap=[[0, P], [1, cols]])

    halves = ((0, 512), (512, 512))

    with tile.TileContext(nc) as tc, ExitStack() as ctx:
        const = ctx.enter_context(tc.tile_pool(name="const", bufs=1))
        work = ctx.enter_context(tc.tile_pool(name="work", bufs=2))
        stats = ctx.enter_context(tc.tile_pool(name="stats", bufs=4))
        h_pool = ctx.enter_context(tc.tile_pool(name="hp", bufs=1))
        at_pool = ctx.enter_context(tc.tile_pool(name="atp", bufs=1))
        out_pool = ctx.enter_context(tc.tile_pool(name="outp", bufs=2))
        wo_po = ctx.enter_context(tc.tile_pool(name="wop", bufs=1))

        # input DMAs: att/w1 on sync, wo/xc/w2 on scalar, vectors on vector
        att_t, wot_t = [], []
        for g in range(2):
            at4 = wo_po.tile([P, 4 * NTOK], BF, tag=f"at{g}", name=f"at{g}")
            nc.sync.dma_start(out=at4[:], in_=at_d[g])
            att_t.append(at4)
            wo4 = wo_po.tile([P, 4 * C], BF, tag=f"wo{g}", name=f"wo{g}")
            nc.scalar.dma_start(out=wo4[:], in_=wo_d[g])
            wot_t.append(wo4)
        xc_t = wo_po.tile([P, NTB * C], BF, tag="xc", name="xc")
        nc.scalar.dma_start(out=xc_t[:], in_=xc_d)

        def attc(c, lo, hi):
            base = (c % 4) * NTOK
            return att_t[c // 4][:, base + lo:base + hi]

        def woc(c, lo, hi):
            base = (c % 4) * C
            return wot_t[c // 4][:, base + lo:base + hi]

        eps_t = const.tile([P, 1], FP)
        nc.vector.memset(eps_t[:], EPS)
        g1b = const.tile([P, C], FP)
        nc.gpsimd.dma_start(out=g1b[:], in_=bcast_rows(g1_d, C))
        g2b = const.tile([P, C], FP)
        nc.gpsimd.dma_start(out=g2b[:], in_=bcast_rows(g2_d, C))
        b2row = const.tile([1, C], BF)
        nc.gpsimd.dma_start(
            out=b2row[:],
            in_=bass.AP(tensor=b2_d.tensor, offset=b2_d.offset,
                        ap=[[0, 1], [1, C]]))
        ones1 = const.tile([1, P], BF)
        nc.vector.memset(ones1[:], 1.0)
        b1s = const.tile([P, NDF], FP)
        nc.gpsimd.dma_start(
            out=b1s[:],
            in_=bass.AP(tensor=b1_d.tensor, offset=b1_d.offset,
                        ap=[[1, P], [P, NDF]]))
        # w1 groups (sync queue), 3-deep window
        w1_po = ctx.enter_context(tc.tile_pool(name="w1p", bufs=3))
        w1g = []
        for g in range(NG):
            t = w1_po.tile([P, NCH * 512], BF, tag="w1g", name="w1g")
            nc.sync.dma_start(out=t[:], in_=w1_d[g])
            w1g.append(t)
        # w2 quads (scalar queue), 3-deep window
        w2_po = ctx.enter_context(tc.tile_pool(name="w2p", bufs=3))
        w2q = []
        for q in range(NDF // 4):
            t = w2_po.tile([P, 4 * C], BF, tag="w2q", name="w2q")
            nc.scalar.dma_start(out=t[:], in_=w2_d[q])
            w2q.append(t)

        def w2c(d, lo, hi):
            base = (d % 4) * C
            return w2q[d // 4][:, base + lo:base + hi]

        junk = const.tile([P, 1], FP)
        nc.vector.memset(junk[:], 1.0)
        junk2 = const.tile([P, 1], FP)
        nc.scalar.activation(junk2[:], junk[:], AF.Sqrt)
        ident = const.tile([P, P], BF)
        make_identity(nc, ident[:])

        hT = h_pool.tile([P, NCH, NTOK], BF, tag="hT")
        h_bfs = []

        # ---- stage 0: o = attn@Wo + x; rmsnorm per token block; -> hT
        with tc.tile_pool(name="o_ps", bufs=1, space="PSUM") as o_ps, \
             tc.tile_pool(name="t_ps", bufs=2, space="PSUM") as t_ps:
            for pair_i in range(NTB // 2):
                pair = (2 * pair_i, 2 * pair_i + 1)
                # 3 rotating tag slots so a fresh pair never waits on the
                # slowest token block of the previous pair
                sl_ids = [(2 * pair_i) % 3, (2 * pair_i + 1) % 3]
                o2 = [[o_ps.tile([P, 512], FP, tag=f"o2_{sl_ids[si]}_{hi}",
                                 name="o2")
                       for hi in range(2)] for si in range(2)]
                for c in range(NCH):
                    for si, tb in enumerate(pair):
                        for hi, (hst, hw) in enumerate(halves):
                            nc.tensor.matmul(
                                o2[si][hi][:],
                                attc(c, tb * P, (tb + 1) * P),
                                woc(c, hst, hst + hw),
                                start=(c == 0), stop=False)
                for si, tb in enumerate(pair):
                    for hi, (hst, hw) in enumerate(halves):
                        nc.tensor.matmul(
                            o2[si][hi][:], ident[:],
                            xc_t[:, tb * C + hst:tb * C + hst + hw],
                            start=False, stop=True)
                for si, tb in enumerate(pair):
                    sq = work.tile([P, 512], FP, tag="sq")
                    ss = [stats.tile([P, 1], FP, tag=f"ss{hi}",
                                     name=f"ss{hi}") for hi in range(2)]
                    for hi in range(2):
                        nc.scalar.activation(sq[:], o2[si][hi][:], AF.Square,
                                             accum_out=ss[hi][:])
                    nc.vector.tensor_add(ss[0][:], ss[0][:], ss[1][:])
                    rstd = stats.tile([P, 1], FP, tag="rstd")
                    nc.scalar.activation(rstd[:], ss[0][:], AF.Sqrt,
                                         scale=1.0 / C, bias=eps_t[:])
                    rinv = stats.tile([P, 1], FP, tag="rinv")
                    nc.vector.reciprocal(rinv[:], rstd[:])
                    h_bf = h_pool.tile([P, C], BF, tag=f"h{tb}")
                    for hi, (hst, hw) in enumerate(halves):
                        nc.vector.scalar_tensor_tensor(
                            h_bf[:, hst:hst + hw], o2[si][hi][:], rinv[:],
                            g1b[:, hst:hst + hw],
                            op0=mybir.AluOpType.mult, op1=mybir.AluOpType.mult)
                    h_bfs.append(h_bf)
                    for g4 in range(2):
                        tp = t_ps.tile([P, 4, P], BF, tag="tp")
                        for u in range(4):
                            nc.tensor.transpose(
                                tp[:, u, :],
                                h_bf[:, (g4 * 4 + u) * P:(g4 * 4 + u + 1) * P],
                                ident[:])
                        nc.vector.tensor_copy(
                            hT[:, g4 * 4:(g4 + 1) * 4, tb * P:(tb + 1) * P],
                            tp[:])

        # ---- stage 1: aT = silu(W1^T @ h^T + b1) via scalar-engine silu
        ats = []
        with tc.tile_pool(name="a_ps", bufs=2, space="PSUM") as a_ps:
            for g in range(NG):
                aps = [a_ps.tile([P, NTOK], FP, tag=f"a{u}", name=f"a{u}")
                       for u in range(4)]
                for c in range(NCH):
                    for u in range(4):
                        nc.tensor.matmul(
                            aps[u][:],
                            w1g[g][:, c * 512 + u * P:c * 512 + (u + 1) * P],
                            hT[:, c, :],
                            start=(c == 0), stop=(c == NCH - 1))
                for u in range(4):
                    d = 4 * g + u
                    at_t = at_pool.tile([P, NTOK], BF, tag=f"at{d}")
                    nc.scalar.activation(at_t[:], aps[u][:], AF.Silu,
                                         bias=b1s[:, d:d + 1], scale=1.0)
                    ats.append(at_t)

        # ---- stage 2: f = aT^T @ W2 + h + b2; rmsnorm + store per block
        with tc.tile_pool(name="f_ps", bufs=1, space="PSUM") as f_ps:
            f2 = [f_ps.tile([P, C], FP, tag=f"f{tb}", name=f"f{tb}")
                  for tb in range(NTB)]
            for tb in range(NTB):
                for (hst, hw) in halves:
                    nc.tensor.matmul(
                        f2[tb][:, hst:hst + hw], ident[:],
                        h_bfs[tb][:, hst:hst + hw],
                        start=True, stop=False)
                    nc.tensor.matmul(
                        f2[tb][:, hst:hst + hw], ones1[:],
                        b2row[:, hst:hst + hw],
                        start=False, stop=False)
            for d in range(NDF - STAG):
                for tb in range(NTB):
                    for (hst, hw) in halves:
                        nc.tensor.matmul(
                            f2[tb][:, hst:hst + hw],
                            ats[d][:, tb * P:(tb + 1) * P],
                            w2c(d, hst, hst + hw),
                            start=False, stop=False)
            for tb in range(NTB):
                for d in range(NDF - STAG, NDF):
                    for (hst, hw) in halves:
                        nc.tensor.matmul(
                            f2[tb][:, hst:hst + hw],
                            ats[d][:, tb * P:(tb + 1) * P],
                            w2c(d, hst, hst + hw),
                            start=False, stop=(d == NDF - 1))
                sq = work.tile([P, C], FP, tag="sq2")
                ssum = stats.tile([P, 1], FP, tag="ssum2")
                nc.scalar.activation(sq[:], f2[tb][:], AF.Square,
                                     accum_out=ssum[:])
                rstd = stats.tile([P, 1], FP, tag="rstd2")
                nc.scalar.activation(rstd[:], ssum[:], AF.Sqrt,
                                     scale=1.0 / C, bias=eps_t[:])
                rinv = stats.tile([P, 1], FP, tag="rinv2")
                nc.vector.reciprocal(rinv[:], rstd[:])
                o = out_pool.tile([P, C], BF, tag="outt")
                nc.vector.scalar_tensor_tensor(
                    o[:], f2[tb][:], rinv[:], g2b[:],
                    op0=mybir.AluOpType.mult, op1=mybir.AluOpType.mult)
                nc.gpsimd.dma_start(
                    out=out_d[tb * P:(tb + 1) * P, :], in_=o[:])
    nc.compile()
    return nc


# --------------------------------------------------------------------------
# host orchestration
# --------------------------------------------------------------------------

_CACHE = {}


def _phase1(B, T, C, DH):
    key = ("p1", B, T, C, DH)
    if key not in _CACHE:
        _CACHE[key] = build_phase1(B, T, C, DH)
    return _CACHE[key]


def _phase2(NTOK, C, DFF):
    key = ("p2", NTOK, C, DFF)
    if key not in _CACHE:
        _CACHE[key] = build_phase2(NTOK, C, DFF)
    return _CACHE[key]


def _run(nc, in_maps):
    import os
    trace = bool(os.environ.get("KERNEL_TRACE"))
    res = run_bass_kernel_spmd(nc, in_maps, core_ids=list(range(N_CORES)),
                               trace=trace)
    LAST_EXEC_NS.append(res.exec_time_ns)
    LAST_TRACES.append(res.instructions_and_trace)
    return res.results


def _pack_rows(a, nrow):
    """[R, W] -> [P, (R//P//nrow groups)...]: group rows so each DMA tile
    [P, nrow*W] is contiguous: out[g, p, i, :] = a[(g*nrow+i)*P + p, :]."""
    R, W = a.shape
    ng = R // (P * nrow)
    return np.ascontiguousarray(
        a.reshape(ng, nrow, P, W).transpose(0, 2, 1, 3).reshape(
            ng, P, nrow * W))


def kernel(x, Wq, Wk, Wv, Wo, bo, W1, b1, W2, b2, g1, g2):
    f32 = lambda a: np.ascontiguousarray(np.asarray(a), dtype=np.float32)
    x = f32(x)
    Wq, Wk, Wv, Wo, bo = f32(Wq), f32(Wk), f32(Wv), f32(Wo), f32(bo)
    W1, b1, W2, b2, g1, g2 = f32(W1), f32(b1), f32(W2), f32(b2), f32(g1), f32(g2)

    B, T, C = x.shape
    H, _, DH = Wq.shape
    HP = H // N_CORES
    DA = DH + 1
    NCC = C // P
    DW = HP * DH
    LAST_EXEC_NS.clear()
    LAST_TRACES.clear()

    # ---- phase 1
    nc1 = _phase1(B, T, C, DH)
    xT = x.transpose(0, 2, 1).astype(BF_NP)            # [B, C, T]
    # pack x: [B, 4, P, 2T] with (b, cp, p, i*T+t) = xT[b, (2cp+i)P+p, t]
    xP = np.ascontiguousarray(
        xT.reshape(B, NCC // 2, 2, P, T).transpose(0, 1, 3, 2, 4).reshape(
            B, NCC // 2, P, 2 * T))
    in1 = []
    for i in range(N_CORES):
        ws = {}
        for nm, W_ in (("wq", Wq), ("wk", Wk), ("wv", Wv)):
            pw = W_[HP * i:HP * (i + 1)].transpose(1, 0, 2).reshape(C, DW)
            ws[nm] = _pack_rows(pw.astype(BF_NP), NCC)[0]
        in1.append({"xT": xP, **ws})
    res1 = _run(nc1, in1)

    attn = np.empty((B, T, C), np.float32)
    for i in range(N_CORES):
        ot = res1[i]["ot"].astype(np.float32)          # [B, HP, DA, T]
        o = ot[:, :, :DH, :]
        den = ot[:, :, DH, :]
        on = o / den[:, :, None, :]
        for hh in range(HP):
            hcol = (HP * i + hh) * DH
            attn[:, :, hcol:hcol + DH] = on[:, hh].transpose(0, 2, 1)

    # ---- phase 2
    NTOK = B * T // N_CORES
    DFF = W1.shape[1]
    NTB = NTOK // P
    nc2 = _phase2(NTOK, C, DFF)
    xf = (x.reshape(B * T, C) + bo).astype(BF_NP)
    af = attn.reshape(B * T, C)
    NCH = C // P
    NG = DFF // 512
    # w1P[g][p, c*512+f] = W1[c*128+p, g*512+f]
    w1P = np.ascontiguousarray(
        W1.astype(BF_NP).reshape(NCH, P, NG, 512).transpose(2, 1, 0, 3)
        .reshape(NG, P, NCH * 512))
    w2P = _pack_rows(W2.astype(BF_NP), 4)              # [8, P, 4*C]
    in2 = []
    for k in range(N_CORES):
        sl = slice(k * NTOK, (k + 1) * NTOK)
        atT = np.ascontiguousarray(af[sl].T).astype(BF_NP)   # [C, NTOK]
        in2.append({
            "xc": _pack_rows(xf[sl], NTB)[0],
            "attnT": _pack_rows(atT, 4),
            "wo": _pack_rows(Wo.astype(BF_NP), 4),
            "w1": w1P, "w2": w2P,
            "g1": g1, "g2": g2, "b1": b1, "b2": b2.astype(BF_NP),
        })
    res2 = _run(nc2, in2)
    out = np.concatenate(
        [res2[k]["out"].astype(np.float32) for k in range(N_CORES)], axis=0)
    return out.reshape(B, T, C)
